# revision 1
# baseline (speedup 1.0000x reference)
import os
import numpy as np
import ml_dtypes
from contextlib import ExitStack
import concourse.bass as bass
import concourse.mybir as mybir
from concourse.ap import AP as APc
from concourse.bass_utils import run_bass_kernel_spmd

B, H, W = 8, 160, 256
C, K = 700, 250
NB = 750
NH, NJ = 30, 60
NJP = 64          # NJ padded for DoubleRow weight step%16
NF = 90
TO = 500          # loss time bins (gen_sig[:, :-1])
P = H * W         # 40960
MAGIC = 400.0 / 750.0

GB, GC = 4, 2     # batch groups x cell groups = 8 cores
BL = B // GB      # 2 batches per core
CL = C // GC      # 350 cells per core
PT = 128
NPT = P // PT     # 320
QPC = 16          # pixel tiles per DMA chunk
NCH = NPT // QPC  # 40
CHW_F = QPC * CL  # 2800
CHW_J = QPC * NJP # 512
CHW = CHW_F + BL * CHW_J  # 3824 fused ft+jt chunk cols
CT_OFF = [0, 128, 256]
CT_SZ = [128, 128, CL - 256]
NPAIR = K // 2    # 125 tap pairs per diag set
FSC = 64.0        # host scale on spatial filters (fp8 subnormal dodge)
TSC = 8.0         # host scale on timecourse filters
F32 = mybir.dt.float32
BF16 = mybir.dt.bfloat16
FP8 = mybir.dt.float8e4
BF = ml_dtypes.bfloat16
F8 = ml_dtypes.float8_e4m3fn
DR = mybir.MatmulPerfMode.DoubleRow


def _build_nc():
    CPY = mybir.ActivationFunctionType.Copy
    EXP = mybir.ActivationFunctionType.Exp
    MU = mybir.AluOpType.mult
    AD = mybir.AluOpType.add
    nc = bass.Bass()
    fjc = nc.dram_tensor("fjc", (NCH, PT, CHW), FP8, kind="ExternalInput")
    hist = nc.dram_tensor("hist", (BL, NH, CL), BF16, kind="ExternalInput")
    Mmat = nc.dram_tensor("Mmat", (BL, NF, NB), BF16, kind="ExternalInput")
    fbp = nc.dram_tensor("fbp", (128, 2 * 3, TO), F32, kind="ExternalInput")
    spnp = nc.dram_tensor("spnp", (128, 2 * 3, TO), F32, kind="ExternalInput")
    mvbp = nc.dram_tensor("mvbp", (128, BL, TO), F32, kind="ExternalInput")
    tfilt = nc.dram_tensor("tfilt", (CL, K), F32, kind="ExternalInput")
    ident = nc.dram_tensor("ident", (128, 128), BF16, kind="ExternalInput")
    part = nc.dram_tensor("part", (BL, CL), F32, kind="ExternalOutput")

    N_PRO = 10
    es = ExitStack()
    with es:
        ident_sb = es.enter_context(nc.sbuf_tensor("ident_sb", [128, 128], BF16))
        tf0 = es.enter_context(nc.sbuf_tensor("tf0", [128, K], F32))
        tf1 = es.enter_context(nc.sbuf_tensor("tf1", [128, K], F32))
        tf2 = es.enter_context(nc.sbuf_tensor("tf2", [CT_SZ[2], K], F32))
        hist0 = es.enter_context(nc.sbuf_tensor("hist0", [NH, CL], BF16))
        hist1 = es.enter_context(nc.sbuf_tensor("hist1", [NH, CL], BF16))
        mh0 = es.enter_context(nc.sbuf_tensor("mh0", [NH, NB], BF16))
        mh1 = es.enter_context(nc.sbuf_tensor("mh1", [NH, NB], BF16))
        mj0 = es.enter_context(nc.sbuf_tensor("mj0", [NJ, NB], BF16))
        mj1 = es.enter_context(nc.sbuf_tensor("mj1", [NJ, NB], BF16))
        mvb_sb = es.enter_context(nc.sbuf_tensor("mvb_sb", [128, BL, TO], F32))
        fb_sb = es.enter_context(nc.sbuf_tensor("fb_sb", [128, 6, TO], F32))
        spn_sb = es.enter_context(nc.sbuf_tensor("spn_sb", [128, 6, TO], F32))
        fj_sb = es.enter_context(nc.sbuf_tensor("fj_sb", [PT, 4, CHW], FP8))
        spatj0 = es.enter_context(nc.sbuf_tensor("spatj0", [NJ, CL], BF16))
        spatj1 = es.enter_context(nc.sbuf_tensor("spatj1", [NJ, CL], BF16))
        up_sb = es.enter_context(nc.sbuf_tensor("up_sb", [128, 4, 2, NB], FP8))
        dg_sb = es.enter_context(nc.sbuf_tensor("dg_sb", [128, 3, NPAIR, 2, 128], FP8))
        gen_sb = es.enter_context(nc.sbuf_tensor("gen_sb", [128, BL, TO], F32))
        tmpa = es.enter_context(nc.sbuf_tensor("tmpa", [128, BL, TO], F32))
        tmpb = es.enter_context(nc.sbuf_tensor("tmpb", [128, BL, TO], F32))
        junk_sb = es.enter_context(nc.sbuf_tensor("junk_sb", [128, 4, TO], F32))
        r1_sb = es.enter_context(nc.sbuf_tensor("r1_sb", [128, 10], F32))
        r2_sb = es.enter_context(nc.sbuf_tensor("r2_sb", [128, 10], F32))
        res_sb = es.enter_context(nc.sbuf_tensor("res_sb", [128, 6], F32))
        spat_ps0 = es.enter_context(nc.psum_tensor("spat_ps0", [NJ, CL], F32))
        spat_ps1 = es.enter_context(nc.psum_tensor("spat_ps1", [NJ, CL], F32))
        up_ps = es.enter_context(nc.psum_tensor("up_ps", [128, 4, 512], F32))
        acc_ps = es.enter_context(nc.psum_tensor("acc_ps", [128, 2, 512], F32))
        prosem = es.enter_context(nc.semaphore("prosem"))
        dsm = [es.enter_context(nc.semaphore(f"dsm{i}")) for i in range(4)]
        psem = es.enter_context(nc.semaphore("psem"))
        scsem = es.enter_context(nc.semaphore("scsem"))
        upsem = es.enter_context(nc.semaphore("upsem"))
        ucsem = es.enter_context(nc.semaphore("ucsem"))
        dgsem0 = es.enter_context(nc.semaphore("dgsem0"))
        dgs0b = es.enter_context(nc.semaphore("dgs0b"))
        dgsem1 = es.enter_context(nc.semaphore("dgsem1"))
        dgsem2 = es.enter_context(nc.semaphore("dgsem2"))
        chsem = es.enter_context(nc.semaphore("chsem"))
        gsem = es.enter_context(nc.semaphore("gsem"))
        tsem = es.enter_context(nc.semaphore("tsem"))
        asem = es.enter_context(nc.semaphore("asem"))
        latsem = es.enter_context(nc.semaphore("latsem"))
        rsem = es.enter_context(nc.semaphore("rsem"))
        osem = es.enter_context(nc.semaphore("osem"))
        block = es.enter_context(nc.Block())

        hist_sb = [hist0, hist1]
        mh_sb = [mh0, mh1]
        mj_sb = [mj0, mj1]
        spatj_sb = [spatj0, spatj1]
        spat_ps = [spat_ps0, spat_ps1]
        tf_sb = [tf0, tf1, tf2]
        dgsem = [dgsem0, dgsem1, dgsem2]


        @block.sync
        def _(sync):
            for ch in range(4):
                sync.dma_start(fj_sb[:, ch], fjc[ch]).then_inc(dsm[ch], 16)
            # spatial streaming, 4-slot buffered fused chunks
            for ch in range(4, NCH):
                sync.wait_ge(psem, ch - 3)
                o = ch % 4
                sync.dma_start(fj_sb[:, o], fjc[ch]).then_inc(dsm[o], 16)
            sync.dma_start(mvb_sb[:], mvbp[:]).then_inc(latsem, 16)
            sync.dma_start(fb_sb[:], fbp[:]).then_inc(latsem, 16)
            sync.dma_start(spn_sb[:], spnp[:]).then_inc(latsem, 16)
            # outputs
            for ct in range(3):
                mc = CT_SZ[ct]
                sync.wait_ge(rsem, ct + 1)
                for b in range(BL):
                    j = 2 * ct + b
                    sync.dma_start(part[b, CT_OFF[ct]:CT_OFF[ct] + mc],
                                   res_sb[0:mc, j:j + 1]).then_inc(osem, 16)
            sync.wait_ge(osem, 16 * 6)

        @block.tensor
        def _(tensor):
            fj_h = fj_sb[0:PT, 0, 0:1]
            ROW = 4 * CHW

            def ft_ap(o, q):
                return APc(fj_h.tensor, o * CHW + q * CL,
                           [[ROW, PT], [CL, 2], [1, CL]])

            def jt_ap(o, b, q):
                return APc(fj_h.tensor, o * CHW + CHW_F + b * CHW_J + q * NJP,
                           [[ROW, PT], [NJP, 2], [1, NJ]])

            for ch in range(NCH):
                o = ch % 4
                tensor.wait_ge(dsm[o], 16 * (ch // 4 + 1))
                mm = None
                for q in range(0, QPC, 2):
                    for b in range(BL):
                        mm = tensor.matmul(spat_ps[b][:, :],
                                           jt_ap(o, b, q),
                                           ft_ap(o, q),
                                           start=(ch == 0 and q == 0),
                                           stop=(ch == NCH - 1 and q == QPC - 2),
                                           skip_group_check=True,
                                           perf_mode=DR)
                mm.then_inc(psem, 1)

            tensor.wait_ge(prosem, 16 * N_PRO)
            tensor.wait_ge(scsem, 1)

            def emit_upsample(ct):
                mc, off = CT_SZ[ct], CT_OFF[ct]
                mm = None
                for b in range(BL):
                    for ch2 in range(2):
                        tensor.matmul(up_ps[0:mc, 2 * b + ch2, 0:375],
                                      hist_sb[b][:, off:off + mc],
                                      mh_sb[b][:, ch2 * 375:(ch2 + 1) * 375],
                                      start=True, stop=False, skip_group_check=True)
                        mm = tensor.matmul(up_ps[0:mc, 2 * b + ch2, 0:375],
                                           spatj_sb[b][:, off:off + mc],
                                           mj_sb[b][:, ch2 * 375:(ch2 + 1) * 375],
                                           start=False, stop=True, skip_group_check=True)
                mm.then_inc(upsem, 1)

            emit_upsample(0)
            for ct in range(3):
                mc = CT_SZ[ct]
                us = ct % 2
                if ct + 1 < 3:
                    tensor.wait_ge(ucsem, ct + 1)   # up_ps free again
                    emit_upsample(ct + 1)
                tensor.wait_ge(ucsem, ct + 1)       # up_sb[us] ready
                tensor.wait_ge(dgsem[ct], 1)
                halves = (((None, 0, 500),) if ct < 2 else
                          ((0, 250, 250), (None, 0, 125), (2, 125, 125)))
                for L, c0, cw in halves:
                    if ct >= 1 and L is None and c0 == 0:
                        tensor.wait_ge(gsem, ct)    # acc_ps consumed
                    for p in range(NPAIR):
                        if ct == 0 and p == NPAIR // 2:
                            tensor.wait_ge(dgs0b, 1)
                        for b in range(BL):
                            if L is not None:
                                ops = up_ps[0:mc, L + b, 0:cw]
                            else:
                                ops = acc_ps[0:mc, b, c0:c0 + cw]
                            mm = tensor.matmul(ops,
                                               dg_sb[0:mc, ct, p, 0:2, 0:mc],
                                               up_sb[0:mc, 2 * us + b, 0:2,
                                                     2 * p + c0:2 * p + c0 + cw],
                                               start=(p == 0), stop=(p == NPAIR - 1),
                                               skip_group_check=True,
                                               perf_mode=DR)
                    mm.then_inc(chsem, 1)

        @block.vector
        def _(vector):
            vector.wait_ge(prosem, 16 * N_PRO)
            mm = None
            for p in range(NPAIR // 2, NPAIR):
                for ko in range(2):
                    col = 2 * p + ko
                    mm = vector.tensor_scalar_mul(dg_sb[0:128, 0, p, ko, 0:128],
                                                  ident_sb[0:128, 0:128],
                                                  tf0[0:128, col:col + 1])
            mm.then_inc(dgs0b, 1)
            mm = None
            for p in range(NPAIR):
                for ko in range(2):
                    col = 2 * p + ko
                    mm = vector.tensor_scalar_mul(dg_sb[0:128, 1, p, ko, 0:128],
                                                  ident_sb[0:128, 0:128],
                                                  tf1[0:128, col:col + 1])
            mm.then_inc(dgsem1, 1)
            vector.wait_ge(latsem, 48)
            ng = 0
            nt = 0
            for ct in range(3):
                mc = CT_SZ[ct]
                halves = (((None, 0, 500),) if ct < 2 else
                          ((0, 250, 250), (None, 0, 125), (2, 125, 125)))
                for hi, (L, c0, cw) in enumerate(halves):
                    ng += 1
                    nt += 2
                    vector.wait_ge(chsem, ng)         # conv (half-)tile done
                    if ct >= 1 and hi == 0:
                        vector.wait_ge(tsem, 2 * ct)  # drain own tmpb read of prev tile
                    acc_src = (up_ps[0:mc, L:L + 2, 0:cw] if L is not None else
                               acc_ps[0:mc, 0:2, c0:c0 + cw])
                    vector.scalar_tensor_tensor(gen_sb[0:mc, 0:2, c0:c0 + cw],
                                                acc_src,
                                                1.0 / TSC,
                                                fb_sb[0:mc, 2 * ct:2 * ct + 2,
                                                      c0:c0 + cw],
                                                MU, AD).then_inc(gsem, 1)
                    vector.wait_ge(gsem, ng)          # drain gen write
                    if hi == 0:
                        vector.wait_ge(asem, ct)      # tmpa/tmpb free
                    vector.tensor_add(tmpa[0:mc, 0:2, c0:c0 + cw],
                                      gen_sb[0:mc, 0:2, c0:c0 + cw],
                                      mvb_sb[0:mc, 0:2, c0:c0 + cw]).then_inc(tsem, 1)
                    vector.tensor_mul(tmpb[0:mc, 0:2, c0:c0 + cw],
                                      gen_sb[0:mc, 0:2, c0:c0 + cw],
                                      spn_sb[0:mc, 2 * ct:2 * ct + 2,
                                             c0:c0 + cw]).then_inc(tsem, 1)
                    if ct >= 1 and hi == 0:
                        pm = CT_SZ[ct - 1]
                        j0 = 2 * (ct - 1)
                        vector.tensor_add(res_sb[0:pm, j0:j0 + 2],
                                          r1_sb[0:pm, j0:j0 + 2],
                                          r2_sb[0:pm, j0:j0 + 2]).then_inc(rsem, 1)
            m2 = CT_SZ[2]
            vector.wait_ge(asem, 5)
            vector.tensor_add(r1_sb[0:m2, 4:6], r1_sb[0:m2, 4:6],
                              r1_sb[0:m2, 6:8]).then_inc(tsem, 1)
            vector.tensor_add(r2_sb[0:m2, 4:6], r2_sb[0:m2, 4:6],
                              r2_sb[0:m2, 6:8]).then_inc(tsem, 1)
            vector.wait_ge(tsem, 12)
            vector.tensor_add(r1_sb[0:m2, 4:6], r1_sb[0:m2, 4:6],
                              r1_sb[0:m2, 8:10]).then_inc(tsem, 1)
            vector.tensor_add(r2_sb[0:m2, 4:6], r2_sb[0:m2, 4:6],
                              r2_sb[0:m2, 8:10]).then_inc(tsem, 1)
            vector.wait_ge(tsem, 14)                  # drain own r1/r2 merges
            vector.tensor_add(res_sb[0:m2, 4:6], r1_sb[0:m2, 4:6],
                              r2_sb[0:m2, 4:6]).then_inc(rsem, 1)

        @block.gpsimd
        def _(gp):
            gp.wait_ge(prosem, 16 * N_PRO)
            mm = None
            for p in range(NPAIR // 2):
                for ko in range(2):
                    col = 2 * p + ko
                    mm = gp.tensor_scalar_mul(dg_sb[0:128, 0, p, ko, 0:128],
                                              ident_sb[0:128, 0:128],
                                              tf0[0:128, col:col + 1])
            mm.then_inc(dgsem0, 1)
            m = CT_SZ[2]
            mm = None
            for p in range(NPAIR):
                for ko in range(2):
                    col = 2 * p + ko
                    mm = gp.tensor_scalar_mul(dg_sb[0:m, 2, p, ko, 0:m],
                                              ident_sb[0:m, 0:m],
                                              tf2[0:m, col:col + 1])
            mm.then_inc(dgsem2, 1)

        @block.scalar
        def _(scalar):
            scalar.dma_start(ident_sb[:], ident[:]).then_inc(prosem, 16)
            for ct in range(3):
                scalar.dma_start(tf_sb[ct][0:CT_SZ[ct], :],
                                 tfilt[CT_OFF[ct]:CT_OFF[ct] + CT_SZ[ct], :]).then_inc(prosem, 16)
            for b in range(BL):
                scalar.dma_start(hist_sb[b][:], hist[b]).then_inc(prosem, 16)
            for b in range(BL):
                scalar.dma_start(mh_sb[b][:], Mmat[b, 0:NH]).then_inc(prosem, 16)
            for b in range(BL):
                scalar.dma_start(mj_sb[b][:], Mmat[b, NH:NF]).then_inc(prosem, 16)
            scalar.wait_ge(prosem, 16 * N_PRO)
            scalar.wait_ge(psem, NCH)
            scalar.mul(spatj_sb[0][:, :], spat_ps0[:, :], 1.0 / FSC)
            scalar.mul(spatj_sb[1][:, :], spat_ps1[:, :], 1.0 / FSC).then_inc(scsem, 1)

            def up_copy(ct):
                mc = CT_SZ[ct]
                us = ct % 2
                bs = (0, 1)
                scalar.wait_ge(upsem, ct + 1)
                if ct >= 2:
                    scalar.wait_ge(chsem, ct - 1)  # conv(ct-2) done with slot
                mm = None
                for b in bs:
                    lane = 2 * us + b
                    scalar.activation(up_sb[0:mc, lane, 0, 0:750],
                                      up_ps[0:mc, 2 * b:2 * b + 2, 0:375], CPY)
                    # r=1 plane: up shifted left by one bin
                    scalar.activation(up_sb[0:mc, lane, 1, 0:374],
                                      up_ps[0:mc, 2 * b, 1:375], CPY)
                    mm = scalar.activation(up_sb[0:mc, lane, 1, 374:749],
                                           up_ps[0:mc, 2 * b + 1, 0:375], CPY)
                mm.then_inc(ucsem, 1)

            up_copy(0)
            up_copy(1)
            nt = 0
            na = 0
            for ct in range(3):
                mc = CT_SZ[ct]
                halves = (((None, 0, 500),) if ct < 2 else
                          ((0, 250, 250), (None, 0, 125), (2, 125, 125)))
                for hi, (L, c0, cw) in enumerate(halves):
                    jc = 2 * ct + 2 * hi
                    nt += 2
                    na += 1
                    scalar.wait_ge(tsem, nt - 1)
                    if na >= 2:
                        scalar.wait_ge(asem, na - 1)  # drain own junk writes
                    scalar.activation(junk_sb[0:mc, 0, 0:cw], tmpa[0:mc, 0, c0:c0 + cw],
                                      EXP, accum_out=r1_sb[0:mc, jc:jc + 1])
                    scalar.activation(junk_sb[0:mc, 1, 0:cw], tmpa[0:mc, 1, c0:c0 + cw],
                                      EXP, accum_out=r1_sb[0:mc, jc + 1:jc + 2])
                    scalar.wait_ge(tsem, nt)
                    scalar.activation(junk_sb[0:mc, 2, 0:cw], tmpb[0:mc, 0, c0:c0 + cw],
                                      CPY, accum_out=r2_sb[0:mc, jc:jc + 1])
                    scalar.activation(junk_sb[0:mc, 3, 0:cw], tmpb[0:mc, 1, c0:c0 + cw],
                                      CPY, accum_out=r2_sb[0:mc, jc + 1:jc + 2]).then_inc(asem, 1)
                    if ct == 0:
                        up_copy(2)
    return nc


_NC_CACHE = {}


def _host_prep(inputs):
    img = np.asarray(inputs["batched_image"], dtype=np.float32)
    spikes = np.asarray(inputs["batched_spikes"], dtype=np.float32)
    em = np.asarray(inputs["eye_movements"]).astype(np.int64)
    tmask = np.asarray(inputs["time_mask"], dtype=np.float32)
    sel = np.asarray(inputs["forward_sel"]).astype(np.int64)
    fw = np.asarray(inputs["forward_weights"], dtype=np.float32)
    F = np.asarray(inputs["stacked_flat_spat_filters"], dtype=np.float32)
    tcf = np.asarray(inputs["stacked_timecourse_filters"], dtype=np.float32)
    fbg = np.asarray(inputs["precomputed_feedback_gensig"], dtype=np.float32)
    histf = np.asarray(inputs["precomputed_history_frames"], dtype=np.float32)

    # jitter on host (pure gather, exact)
    jit = np.zeros((B, NJ, H, W), dtype=np.float32)
    for b in range(B):
        for f in range(NJ):
            dy, dx = int(em[b, f, 0]), int(em[b, f, 1])
            ys, xs = max(0, -dy), max(0, -dx)
            ye, xe = min(H, H - dy), min(W, W - dx)
            if ye > ys and xe > xs:
                jit[b, f, ys:ye, xs:xe] = img[b, ys + dy:ye + dy, xs + dx:xe + dx]
    jitT = jit.reshape(B, NJ, P).transpose(0, 2, 1)                 # (B,P,NJ)
    jitp = np.zeros((B, NCH, PT, QPC, NJP), dtype=np.float32)
    jitp[..., 0:NJ] = jitT.reshape(B, NCH, QPC, PT, NJ).transpose(0, 1, 3, 2, 4)
    jitc = np.ascontiguousarray(jitp.reshape(B, NCH, PT, CHW_J)).astype(F8)
    jit2_h = [np.stack([jitc[BL * bg + b] for b in range(BL)], axis=2)
              .reshape(NCH, PT, BL * CHW_J) for bg in range(GB)]

    FTf = F.T * np.float32(FSC)                                     # (P,C)
    ftc_h = []
    for cg in range(GC):
        X = FTf[:, cg * CL:(cg + 1) * CL]
        ftc_h.append(np.ascontiguousarray(
            X.reshape(NCH, QPC, PT, CL).transpose(0, 2, 1, 3)
            .reshape(NCH, PT, CHW_F)).astype(F8))

    # upsample mixing matrix M[f,t]
    Mm = np.zeros((B, NF, NB), dtype=np.float32)
    tix = np.arange(NB)
    for b in range(B):
        np.add.at(Mm[b], (sel[b, :, 0], tix), fw[b, :, 0])
        np.add.at(Mm[b], (sel[b, :, 1], tix), fw[b, :, 1])
    Mmb = Mm.astype(BF)

    mv = tmask * np.float32(MAGIC)                                  # (B,500)
    with np.errstate(divide="ignore"):
        lmv = np.log(mv).astype(np.float32)
    spn_all = -(spikes[:, :, K:] * mv[:, None, :])                  # (B,C,500)
    fb5 = fbg[:, :, :TO]
    histb = histf.astype(BF)                                        # (B,NH,C)
    identity = np.eye(128, dtype=np.float32).astype(BF)
    tcf_s = tcf * np.float32(TSC)

    in_maps = []
    for i in range(8):
        bg, cg = i // GC, i % GC
        bs = slice(BL * bg, BL * (bg + 1))
        cs = slice(CL * cg, CL * (cg + 1))
        fbp = np.zeros((128, 6, TO), dtype=np.float32)
        spnp = np.zeros((128, 6, TO), dtype=np.float32)
        for ct in range(3):
            mc, off = CT_SZ[ct], CT_OFF[ct]
            for b in range(BL):
                fbp[0:mc, 2 * ct + b, :] = fb5[BL * bg + b, cg * CL + off:cg * CL + off + mc, :]
                spnp[0:mc, 2 * ct + b, :] = spn_all[BL * bg + b, cg * CL + off:cg * CL + off + mc, :]
        mvbp = np.ascontiguousarray(np.broadcast_to(
            lmv[bs][None, :, :], (128, BL, TO)))
        in_maps.append({
            "fjc": np.ascontiguousarray(
                np.concatenate([ftc_h[cg], jit2_h[bg]], axis=2)),
            "hist": np.ascontiguousarray(histb[bs][:, :, cs]),
            "Mmat": Mmb[bs],
            "tfilt": np.ascontiguousarray(tcf_s[cs]),
            "fbp": fbp,
            "spnp": spnp,
            "mvbp": mvbp,
            "ident": identity,
        })
    return in_maps


def kernel(**inputs) -> np.ndarray:
    in_maps = _host_prep(inputs)
    if "nc" not in _NC_CACHE:
        _NC_CACHE["nc"] = _build_nc()
    nc = _NC_CACHE["nc"]

    if os.environ.get("KTRACE"):
        res = run_bass_kernel_spmd(
            nc, in_maps, core_ids=list(range(8)), trace=True,
            trace_cores=[0], tmpdir=os.environ.get("KTRACE_DIR") or None)
        kernel.last_results = res
    else:
        res = run_bass_kernel_spmd(nc, in_maps, core_ids=list(range(8)))
    out = np.zeros(B, dtype=np.float64)
    for i in range(8):
        bg = i // GC
        out[BL * bg:BL * (bg + 1)] += res.results[i]["part"].sum(axis=1, dtype=np.float64)
    return out.astype(np.float32)



# revision 2
# speedup vs baseline: 1.2129x; 1.2129x over previous
import os
import numpy as np
import ml_dtypes
from contextlib import ExitStack
import concourse.bass as bass
import concourse.mybir as mybir
from concourse.ap import AP as APc
from concourse.bass_utils import run_bass_kernel_spmd

B, H, W = 8, 160, 256
C, K = 700, 250
NB = 750
NH, NJ = 30, 60
NJP = 64
NF = 90
TO = 500
P = H * W
MAGIC = 400.0 / 750.0

GB, GC = 4, 2      # batch groups x cell groups
BL = B // GB       # 2 batches per core
CL = C // GC       # 350 cells per core
PT = 128
NPT = P // PT      # 320 pixel tiles
QPC = 16           # pixel tiles per chunk
NCH = NPT // QPC   # 20 chunks
NPC = 5            # DMA pieces per (jt|ft) stream
CPP = NCH // NPC   # 4 chunks per piece
CT_SZ = [94, 128, 128]   # smallest tile first: shortest DMA lead-in
CT_OFF = [0, 94, 222]
NPAIR = K // 2     # 125 tap pairs
NU = 6             # conv units = 3 tiles x 2 batches
FSC = 64.0
TSC = 8.0
F32 = mybir.dt.float32
BF16 = mybir.dt.bfloat16
FP8 = mybir.dt.float8e4
BF = ml_dtypes.bfloat16
F8 = ml_dtypes.float8_e4m3fn
DR = mybir.MatmulPerfMode.DoubleRow

ROW_JT = NCH * QPC * BL * NJP    # 40960 per-partition elems
ROW_FT = NCH * 2048              # ft_sb free width
# diag build split per set: (dve_pairs, pool_pairs, act_pairs)
DG_SPLIT = [(65, 30, 30), (65, 60, 0), (80, 45, 0)]
DG_SLOT = [0, 1, 0]


def _build_nc():
    CPY = mybir.ActivationFunctionType.Copy
    EXP = mybir.ActivationFunctionType.Exp
    MU = mybir.AluOpType.mult
    AD = mybir.AluOpType.add
    nc = bass.Bass()
    jtd = nc.dram_tensor("jtd", (PT, NCH, QPC, BL, NJP), FP8, kind="ExternalInput")
    ft0 = nc.dram_tensor("ft0", (PT, NCH, QPC * CT_SZ[0]), FP8, kind="ExternalInput")
    ft1 = nc.dram_tensor("ft1", (PT, NCH, QPC * CT_SZ[1]), FP8, kind="ExternalInput")
    ft2 = nc.dram_tensor("ft2", (PT, NCH, QPC * CT_SZ[2]), FP8, kind="ExternalInput")
    histd = nc.dram_tensor("histd", (NH, BL, CL), BF16, kind="ExternalInput")
    mhd = nc.dram_tensor("mhd", (NH, BL, NB), BF16, kind="ExternalInput")
    mjd = nc.dram_tensor("mjd", (128, BL, NB), BF16, kind="ExternalInput")
    fbd = nc.dram_tensor("fbd", (3, 128, BL, TO), BF16, kind="ExternalInput")
    spnd = nc.dram_tensor("spnd", (3, 128, BL, TO), BF16, kind="ExternalInput")
    tfd = nc.dram_tensor("tfd", (128, 3, K), F32, kind="ExternalInput")
    ident = nc.dram_tensor("ident", (128, 128), BF16, kind="ExternalInput")
    part = nc.dram_tensor("part", (BL, CL), F32, kind="ExternalOutput")
    ftd = [ft0, ft1, ft2]

    es = ExitStack()
    with es:
        jt_sb = es.enter_context(nc.sbuf_tensor("jt_sb", [PT, NCH, QPC, BL, NJP], FP8))
        ft_sb = es.enter_context(nc.sbuf_tensor("ft_sb", [PT, NCH, 2048], FP8))
        dg_sb = es.enter_context(nc.sbuf_tensor("dg_sb", [128, 2, NPAIR, 2, 128], FP8))
        ident_sb = es.enter_context(nc.sbuf_tensor("ident_sb", [128, 128], BF16))
        tf_sb = es.enter_context(nc.sbuf_tensor("tf_sb", [128, 3, K], F32))
        hist_sb = es.enter_context(nc.sbuf_tensor("hist_sb", [NH, BL, CL], BF16))
        mh_sb = es.enter_context(nc.sbuf_tensor("mh_sb", [NH, BL, NB], BF16))
        mj_sb = es.enter_context(nc.sbuf_tensor("mj_sb", [128, BL, NB], BF16))
        spatj_sb = es.enter_context(nc.sbuf_tensor("spatj_sb", [128, 2, 128], BF16))
        up_sb = es.enter_context(nc.sbuf_tensor("up_sb", [128, 2, BL, 2, 752], FP8))
        fb_sb = es.enter_context(nc.sbuf_tensor("fb_sb", [128, 2, BL, TO], BF16))
        spn_sb = es.enter_context(nc.sbuf_tensor("spn_sb", [128, 2, BL, TO], BF16))
        tmpa_sb = es.enter_context(nc.sbuf_tensor("tmpa_sb", [128, 2, TO], F32))
        tmpb_sb = es.enter_context(nc.sbuf_tensor("tmpb_sb", [128, 2, TO], F32))
        junk_sb = es.enter_context(nc.sbuf_tensor("junk_sb", [128, 2, TO], F32))
        r1_sb = es.enter_context(nc.sbuf_tensor("r1_sb", [128, NU], F32))
        r2_sb = es.enter_context(nc.sbuf_tensor("r2_sb", [128, NU + 1], F32))
        ptmp_sb = es.enter_context(nc.sbuf_tensor("ptmp_sb", [128, 256], F32))
        res_sb = es.enter_context(nc.sbuf_tensor("res_sb", [128, NU], F32))
        spat_ps = es.enter_context(nc.psum_tensor("spat_ps", [128, 128], F32))
        up_ps = es.enter_context(nc.psum_tensor("up_ps", [128, BL, 2, 512], F32))
        acc_ps = es.enter_context(nc.psum_tensor("acc_ps", [128, 3, 512], F32))
        jsems = [es.enter_context(nc.semaphore(f"jsem{i}")) for i in range(NPC)]
        fsems = [es.enter_context(nc.semaphore(f"fsem{i}")) for i in range(NPC)]
        psem = es.enter_context(nc.semaphore("psem"))    # spatial tile done
        scsem = es.enter_context(nc.semaphore("scsem"))  # spat copied to sbuf
        upsem = es.enter_context(nc.semaphore("upsem"))  # upsample done per (ct,b)
        ucsem = es.enter_context(nc.semaphore("ucsem"))  # up copied per (ct,b)
        dgsem = [es.enter_context(nc.semaphore(f"dgsem{i}")) for i in range(3)]
        chsem = es.enter_context(nc.semaphore("chsem"))  # conv unit done
        gsem = es.enter_context(nc.semaphore("gsem"))    # tmpb built (acc free)
        tsem = es.enter_context(nc.semaphore("tsem"))    # tmpa ready
        asem = es.enter_context(nc.semaphore("asem"))    # act accums done
        rsem = es.enter_context(nc.semaphore("rsem"))    # res ready
        osem = es.enter_context(nc.semaphore("osem"))
        dsem = es.enter_context(nc.semaphore("dsem"))    # ident+tf dmas
        hsem = es.enter_context(nc.semaphore("hsem"))    # hist+mh+mj dmas
        fbsems = [es.enter_context(nc.semaphore(f"fbsem{i}")) for i in range(3)]
        block = es.enter_context(nc.Block())

        jt_h = jt_sb[0:PT, 0, 0, 0, 0:1]
        ft_h = ft_sb[0:PT, 0, 0:1]

        def jt_ap(ch, q):
            return APc(jt_h.tensor, ch * 2048 + q * (BL * NJP),
                       [[ROW_JT, PT], [BL * NJP, 2], [NJP, BL], [1, NJP]])

        def ft_ap(ch, q, mc):
            return APc(ft_h.tensor, ch * 2048 + q * mc,
                       [[ROW_FT, PT], [mc, 2], [1, mc]])

        @block.sync
        def _(sync):
            sync.dma_start(ident_sb[:], ident[:]).then_inc(dsem, 16)
            sync.wait_ge(dsem, 16)
            sync.dma_start(tf_sb[:], tfd[:]).then_inc(dsem, 16)
            # lead-in: interleave jt and ft0 pieces
            for i in range(NPC):
                sync.dma_start(jt_sb[:, CPP * i:CPP * (i + 1)],
                               jtd[:, CPP * i:CPP * (i + 1)]).then_inc(jsems[i], 16)
                sync.dma_start(
                    ft_sb[:, CPP * i:CPP * (i + 1), 0:QPC * CT_SZ[0]],
                    ftd[0][:, CPP * i:CPP * (i + 1)]).then_inc(fsems[i], 16)
            sync.dma_start(fb_sb[:, 0], fbd[0]).then_inc(fbsems[0], 16)
            sync.wait_ge(fbsems[0], 16)
            sync.dma_start(spn_sb[:, 0], spnd[0]).then_inc(fbsems[0], 16)
            # ft1 after spatial ct0 released ft_sb
            sync.wait_ge(psem, 1)
            for i in range(NPC):
                sync.wait_ge(fsems[i], 16)
                sync.dma_start(
                    ft_sb[:, CPP * i:CPP * (i + 1), 0:QPC * CT_SZ[1]],
                    ftd[1][:, CPP * i:CPP * (i + 1)]).then_inc(fsems[i], 16)
            sync.dma_start(fb_sb[:, 1], fbd[1]).then_inc(fbsems[1], 16)
            sync.wait_ge(fbsems[1], 16)
            sync.dma_start(spn_sb[:, 1], spnd[1]).then_inc(fbsems[1], 16)
            sync.wait_ge(psem, 2)
            for i in range(NPC):
                sync.wait_ge(fsems[i], 32)
                sync.dma_start(
                    ft_sb[:, CPP * i:CPP * (i + 1), 0:QPC * CT_SZ[2]],
                    ftd[2][:, CPP * i:CPP * (i + 1)]).then_inc(fsems[i], 16)
            # fb slot 0 reused for ct2: wait units 0,1 postproc done
            sync.wait_ge(gsem, 2)
            sync.dma_start(fb_sb[:, 0], fbd[2]).then_inc(fbsems[2], 16)
            sync.wait_ge(fbsems[2], 16)
            sync.dma_start(spn_sb[:, 0], spnd[2]).then_inc(fbsems[2], 16)
            # outputs
            for u in range(NU):
                ct, b = u // 2, u % 2
                mc, off = CT_SZ[ct], CT_OFF[ct]
                sync.wait_ge(rsem, u + 1)
                sync.dma_start(part[b, off:off + mc],
                               res_sb[0:mc, u:u + 1]).then_inc(osem, 16)
            sync.wait_ge(osem, 16 * NU)

        @block.tensor
        def _(tensor):
            def spatial(ct):
                mc = CT_SZ[ct]
                if ct >= 1:
                    tensor.wait_ge(scsem, ct)   # spat_ps drained by copy
                mm = None
                for pc in range(NPC):
                    if ct == 0 and pc == NPC - 1:
                        # keep PE busy ~3us so last burst + upsample run at
                        # full pstate (ramp needs >3us continuous execution)
                        for _ in range(30):
                            tensor.matmul(acc_ps[0:128, 0, 0:128],
                                          ident_sb[0:128, 0:128],
                                          ident_sb[0:128, 0:128],
                                          start=True, stop=True,
                                          skip_group_check=True)
                    tensor.wait_ge(fsems[pc], 16 * (ct + 1))
                    if ct == 0:
                        tensor.wait_ge(jsems[pc], 16)
                    for ch in range(CPP * pc, CPP * (pc + 1)):
                        for q in range(0, QPC, 2):
                            mm = tensor.matmul(
                                spat_ps[0:128, 0:mc],
                                jt_ap(ch, q), ft_ap(ch, q, mc),
                                start=(ch == 0 and q == 0),
                                stop=(ch == NCH - 1 and q == QPC - 2),
                                skip_group_check=True, perf_mode=DR)
                mm.then_inc(psem, 1)

            def upsample(ct):
                mc, off = CT_SZ[ct], CT_OFF[ct]
                if ct == 0:
                    tensor.wait_ge(hsem, 16 * 3)
                tensor.wait_ge(scsem, ct + 1)
                if ct >= 1:
                    tensor.wait_ge(ucsem, 2 * ct)   # up_ps drained
                for b in range(BL):
                    mm = None
                    for h in range(2):
                        tensor.matmul(up_ps[0:mc, b, h, 0:375],
                                      hist_sb[0:NH, b, off:off + mc],
                                      mh_sb[0:NH, b, h * 375:(h + 1) * 375],
                                      start=True, stop=False,
                                      skip_group_check=True)
                        mm = tensor.matmul(
                            up_ps[0:mc, b, h, 0:375],
                            spatj_sb[64 * b:64 * b + 60, ct % 2, 0:mc],
                            mj_sb[64 * b:64 * b + 60, b, h * 375:(h + 1) * 375],
                            start=False, stop=True, skip_group_check=True)
                    mm.then_inc(upsem, 1)

            def conv(u):
                ct, b = u // 2, u % 2
                mc = CT_SZ[ct]
                lane = u % 3
                tensor.wait_ge(dgsem[ct], 2 * NPAIR)
                tensor.wait_ge(ucsem, 2 * ct + b + 1)
                if u >= 3:
                    tensor.wait_ge(gsem, u - 2)
                mm = None
                for p in range(NPAIR):
                    mm = tensor.matmul(
                        acc_ps[0:mc, lane, 0:TO],
                        dg_sb[0:mc, DG_SLOT[ct], p, 0:2, 0:mc],
                        up_sb[0:mc, ct % 2, b, 0:2, 2 * p:2 * p + TO],
                        start=(p == 0), stop=(p == NPAIR - 1),
                        skip_group_check=True, perf_mode=DR)
                mm.then_inc(chsem, 1)

            spatial(0)
            upsample(0)
            conv(0)
            spatial(1)
            upsample(1)
            conv(1)
            spatial(2)
            upsample(2)
            conv(2)
            conv(3)
            conv(4)
            conv(5)

        def dg_build(ct, p0, p1, ts_fn, sem_target):
            mc = CT_SZ[ct]
            for p in range(p0, p1):
                for j in range(2):
                    col = 2 * p + j
                    ts_fn(dg_sb[0:mc, DG_SLOT[ct], p, j, 0:mc],
                          ident_sb[0:mc, 0:mc],
                          tf_sb[0:mc, ct, col:col + 1]).then_inc(sem_target[ct], 1)

        @block.gpsimd
        def _(gp):
            gp.wait_ge(dsem, 16 * 2)
            d0, p0, a0 = DG_SPLIT[0]
            dg_build(0, d0, d0 + p0, gp.tensor_scalar_mul, dgsem)
            d1, p1, a1 = DG_SPLIT[1]
            dg_build(1, d1, d1 + p1, gp.tensor_scalar_mul, dgsem)
            gp.wait_ge(chsem, 2)   # slot 0 free after conv units 0,1
            d2, p2, a2 = DG_SPLIT[2]
            dg_build(2, d2, d2 + p2, gp.tensor_scalar_mul, dgsem)


        @block.vector
        def _(vector):
            vector.wait_ge(dsem, 16 * 2)
            d0, p0, a0 = DG_SPLIT[0]
            dg_build(0, 0, d0, vector.tensor_scalar_mul, dgsem)
            d1, p1, a1 = DG_SPLIT[1]
            dg_build(1, 0, 40, vector.tensor_scalar_mul, dgsem)

            def postproc(u):
                ct, b = u // 2, u % 2
                mc = CT_SZ[ct]
                lane, us = u % 3, u % 2
                fslot = [0, 1, 0][ct]
                vector.wait_ge(chsem, u + 1)
                vector.wait_ge(fbsems[ct], 32)
                if u >= 2:
                    vector.wait_ge(asem, u - 1)   # tmpa slot us free
                vector.scalar_tensor_tensor(
                    tmpa_sb[0:mc, us, :], acc_ps[0:mc, lane, 0:TO],
                    1.0 / TSC, fb_sb[0:mc, fslot, b, :], MU, AD).then_inc(tsem, 1)
                vector.scalar_tensor_tensor(
                    tmpb_sb[0:mc, us, :], acc_ps[0:mc, lane, 0:TO],
                    1.0 / TSC, spn_sb[0:mc, fslot, b, :], MU, MU).then_inc(gsem, 1)

            def res_add(u):
                mc = CT_SZ[u // 2]
                vector.wait_ge(asem, u + 1)
                vector.wait_ge(gsem, u + 1)
                vector.tensor_add(res_sb[0:mc, u:u + 1], r1_sb[0:mc, u:u + 1],
                                  r2_sb[0:mc, u:u + 1]).then_inc(rsem, 1)

            dg_build(1, 40, d1, vector.tensor_scalar_mul, dgsem)
            postproc(0)
            d2, p2, a2 = DG_SPLIT[2]
            vector.wait_ge(chsem, 2)   # slot 0 free
            postproc(1)
            res_add(0)
            dg_build(2, 0, 40, vector.tensor_scalar_mul, dgsem)
            postproc(2)
            res_add(1)
            dg_build(2, 40, d2, vector.tensor_scalar_mul, dgsem)
            postproc(3)
            res_add(2)
            postproc(4)
            res_add(3)
            postproc(5)
            res_add(4)
            res_add(5)

        @block.scalar
        def _(scalar):
            scalar.dma_start(hist_sb[:], histd[:]).then_inc(hsem, 16)
            scalar.wait_ge(hsem, 16)
            scalar.dma_start(mh_sb[:], mhd[:]).then_inc(hsem, 16)
            scalar.wait_ge(hsem, 32)
            scalar.dma_start(mj_sb[:], mjd[:]).then_inc(hsem, 16)
            scalar.wait_ge(dsem, 16 * 2)
            d0, p0, a0 = DG_SPLIT[0]
            if a0:
                dg_build(0, d0 + p0, NPAIR,
                         lambda o, i, s: scalar.mul(o, i, s), dgsem)

            def spat_copy(ct):
                scalar.wait_ge(psem, ct + 1)
                if ct >= 2:
                    scalar.wait_ge(upsem, 2 * (ct - 1))  # spatj slot free
                scalar.mul(spatj_sb[0:128, ct % 2, 0:CT_SZ[ct]],
                           spat_ps[0:128, 0:CT_SZ[ct]],
                           1.0 / FSC).then_inc(scsem, 1)

            def up_copy(ct, b):
                mc = CT_SZ[ct]
                us = ct % 2
                scalar.wait_ge(upsem, 2 * ct + b + 1)
                if ct >= 2 and b == 0:
                    scalar.wait_ge(chsem, 2)  # up_sb slot0 free after units 0,1
                scalar.activation(up_sb[0:mc, us, b, 0, 0:750],
                                  up_ps[0:mc, b, 0:2, 0:375], CPY)
                scalar.activation(up_sb[0:mc, us, b, 1, 0:374],
                                  up_ps[0:mc, b, 0, 1:375], CPY)
                scalar.activation(up_sb[0:mc, us, b, 1, 374:749],
                                  up_ps[0:mc, b, 1, 0:375],
                                  CPY).then_inc(ucsem, 1)

            def accums(u):
                ct, b = u // 2, u % 2
                mc = CT_SZ[ct]
                us = u % 2
                scalar.wait_ge(tsem, u + 1)
                scalar.activation(junk_sb[0:mc, us, :], tmpa_sb[0:mc, us, :],
                                  EXP, accum_out=r1_sb[0:mc, u:u + 1])
                scalar.wait_ge(gsem, u + 1)
                scalar.activation(junk_sb[0:mc, us, :], tmpb_sb[0:mc, us, :],
                                  CPY, accum_out=r2_sb[0:mc, u:u + 1]).then_inc(asem, 1)

            spat_copy(0)
            up_copy(0, 0)
            up_copy(0, 1)
            accums(0)
            spat_copy(1)
            up_copy(1, 0)
            up_copy(1, 1)
            spat_copy(2)
            up_copy(2, 0)
            up_copy(2, 1)
            accums(1)
            accums(2)
            accums(3)
            accums(4)
            accums(5)
    return nc


_NC_CACHE = {}


def _host_prep(inputs):
    img = np.asarray(inputs["batched_image"], dtype=np.float32)
    spikes = np.asarray(inputs["batched_spikes"], dtype=np.float32)
    em = np.asarray(inputs["eye_movements"]).astype(np.int64)
    tmask = np.asarray(inputs["time_mask"], dtype=np.float32)
    sel = np.asarray(inputs["forward_sel"]).astype(np.int64)
    fw = np.asarray(inputs["forward_weights"], dtype=np.float32)
    F = np.asarray(inputs["stacked_flat_spat_filters"], dtype=np.float32)
    tcf = np.asarray(inputs["stacked_timecourse_filters"], dtype=np.float32)
    fbg = np.asarray(inputs["precomputed_feedback_gensig"], dtype=np.float32)
    histf = np.asarray(inputs["precomputed_history_frames"], dtype=np.float32)

    # jitter on host (pure gather, exact)
    jit = np.zeros((B, NJ, H, W), dtype=np.float32)
    for b in range(B):
        for f in range(NJ):
            dy, dx = int(em[b, f, 0]), int(em[b, f, 1])
            ys, xs = max(0, -dy), max(0, -dx)
            ye, xe = min(H, H - dy), min(W, W - dx)
            if ye > ys and xe > xs:
                jit[b, f, ys:ye, xs:xe] = img[b, ys + dy:ye + dy, xs + dx:xe + dx]
    jitT = jit.reshape(B, NJ, P).transpose(0, 2, 1)                 # (B,P,NJ)
    # pixel index = ch*2048 + q*128 + p
    jitp = np.zeros((B, NCH, PT, QPC, NJP), dtype=np.float32)
    jitp[..., 0:NJ] = jitT.reshape(B, NCH, QPC, PT, NJ).transpose(0, 1, 3, 2, 4)
    jitc = jitp.astype(F8)
    # jtd per batch group: (PT, NCH, QPC, BL, NJP)
    jt_h = [np.ascontiguousarray(
        np.stack([jitc[BL * bg + b] for b in range(BL)], axis=3)
        .transpose(1, 0, 2, 3, 4))
        for bg in range(GB)]

    FTf = F.T * np.float32(FSC)                                     # (P,C)
    ftc_h = {}
    for cg in range(GC):
        for ct in range(3):
            mc, off = CT_SZ[ct], CT_OFF[ct]
            X = FTf[:, cg * CL + off:cg * CL + off + mc]
            ftc_h[(cg, ct)] = np.ascontiguousarray(
                X.reshape(NCH, QPC, PT, mc).transpose(2, 0, 1, 3)
                .reshape(PT, NCH, QPC * mc)).astype(F8)

    # upsample mixing matrix M[f,t]
    Mm = np.zeros((B, NF, NB), dtype=np.float32)
    tix = np.arange(NB)
    for b in range(B):
        np.add.at(Mm[b], (sel[b, :, 0], tix), fw[b, :, 0])
        np.add.at(Mm[b], (sel[b, :, 1], tix), fw[b, :, 1])
    Mmb = Mm.astype(BF)

    mv = tmask * np.float32(MAGIC)                                  # (B,500)
    with np.errstate(divide="ignore"):
        lmv = np.log(mv).astype(np.float32)
    spn_all = -(spikes[:, :, K:] * mv[:, None, :])                  # (B,C,500)
    fb5 = fbg[:, :, :TO]
    fbl_all = fb5 + lmv[:, None, :]                                 # fb + log(mv)
    # host part of linear term: sum_{c,t} spn*fb per batch
    hconst = np.einsum('bct,bct->b', spn_all.astype(np.float64), fb5.astype(np.float64))
    histb = histf.astype(BF)                                        # (B,NH,C)
    identity = np.eye(128, dtype=np.float32).astype(BF)
    tcf_s = tcf * np.float32(TSC)

    in_maps = []
    for i in range(8):
        bg, cg = i // GC, i % GC
        bs = slice(BL * bg, BL * (bg + 1))
        cs = slice(CL * cg, CL * (cg + 1))
        fbp = np.zeros((3, 128, BL, TO), dtype=BF)
        spnp = np.zeros((3, 128, BL, TO), dtype=BF)
        for ct in range(3):
            mc, off = CT_SZ[ct], CT_OFF[ct]
            for b in range(BL):
                fbp[ct, 0:mc, b, :] = fbl_all[BL * bg + b,
                                              cg * CL + off:cg * CL + off + mc, :].astype(BF)
                spnp[ct, 0:mc, b, :] = spn_all[BL * bg + b,
                                               cg * CL + off:cg * CL + off + mc, :].astype(BF)
        mjp = np.zeros((128, BL, NB), dtype=BF)
        mjp[0:NJ] = Mmb[bs][:, NH:NF].transpose(1, 0, 2)
        mjp[64:64 + NJ] = mjp[0:NJ]
        tfp = np.zeros((128, 3, K), dtype=np.float32)
        for ct in range(3):
            mc, off = CT_SZ[ct], CT_OFF[ct]
            tfp[0:mc, ct] = tcf_s[cg * CL + off:cg * CL + off + mc]
        in_maps.append({
            "jtd": jt_h[bg],
            "ft0": ftc_h[(cg, 0)],
            "ft1": ftc_h[(cg, 1)],
            "ft2": ftc_h[(cg, 2)],
            "histd": np.ascontiguousarray(histb[bs][:, :, cs].transpose(1, 0, 2)),
            "mhd": np.ascontiguousarray(Mmb[bs][:, 0:NH].transpose(1, 0, 2)),
            "mjd": mjp,
            "fbd": fbp,
            "spnd": spnp,
            "tfd": tfp,
            "ident": identity,
        })
    return in_maps, hconst


def kernel(**inputs) -> np.ndarray:
    in_maps, hconst = _host_prep(inputs)
    if "nc" not in _NC_CACHE:
        _NC_CACHE["nc"] = _build_nc()
    nc = _NC_CACHE["nc"]

    if os.environ.get("KTRACE"):
        res = run_bass_kernel_spmd(
            nc, in_maps, core_ids=list(range(8)), trace=True,
            trace_cores=[0], tmpdir=os.environ.get("KTRACE_DIR") or None)
        kernel.last_results = res
    else:
        res = run_bass_kernel_spmd(nc, in_maps, core_ids=list(range(8)))
    out = np.array(hconst, dtype=np.float64)
    for i in range(8):
        bg = i // GC
        out[BL * bg:BL * (bg + 1)] += res.results[i]["part"].sum(axis=1, dtype=np.float64)
    return out.astype(np.float32)


# revision 5
# speedup vs baseline: 1.2489x; 1.0297x over previous
import os
import numpy as np
import ml_dtypes
from contextlib import ExitStack
import concourse.bass as bass
import concourse.mybir as mybir
from concourse.ap import AP as APc
from concourse.bass_utils import run_bass_kernel_spmd

B, H, W = 8, 160, 256
C, K = 700, 250
NB = 750
NH, NJ = 30, 60
NJP = 64
NF = 90
TO = 500
P = H * W
MAGIC = 400.0 / 750.0

GB, GC = 4, 2      # batch groups x cell groups
BL = B // GB       # 2 batches per core
CL = C // GC       # 350 cells per core
PT = 128
NPT = P // PT      # 320 pixel tiles
QPC = 16           # pixel tiles per chunk
NCH = NPT // QPC   # 20 chunks
NPC = 5            # DMA pieces per (jt|ft) stream
CPP = NCH // NPC   # 4 chunks per piece
CT_SZ = [94, 128, 128]   # smallest tile first: shortest DMA lead-in
CT_OFF = [0, 94, 222]
NPAIR = K // 2     # 125 tap pairs
NU = 6             # conv units = 3 tiles x 2 batches
FSC = 64.0
TSC = 8.0
F32 = mybir.dt.float32
BF16 = mybir.dt.bfloat16
FP8 = mybir.dt.float8e4
BF = ml_dtypes.bfloat16
F8 = ml_dtypes.float8_e4m3fn
DR = mybir.MatmulPerfMode.DoubleRow

ROW_JT = NCH * QPC * BL * NJP    # 40960 per-partition elems
ROW_FT = NCH * 2048              # ft_sb free width
# diag build split per set: (dve_pairs, pool_pairs, act_pairs)
DG_SPLIT = [(65, 30, 30), (65, 60, 0), (80, 45, 0)]
DG_SLOT = [0, 1, 0]


def _build_nc():
    CPY = mybir.ActivationFunctionType.Copy
    EXP = mybir.ActivationFunctionType.Exp
    MU = mybir.AluOpType.mult
    AD = mybir.AluOpType.add
    AXX = mybir.AxisListType.X
    nc = bass.Bass()
    jtd = nc.dram_tensor("jtd", (PT, NCH, QPC, BL, NJP), FP8, kind="ExternalInput")
    ft0 = nc.dram_tensor("ft0", (PT, NCH, QPC * CT_SZ[0]), FP8, kind="ExternalInput")
    ft1 = nc.dram_tensor("ft1", (PT, NCH, QPC * CT_SZ[1]), FP8, kind="ExternalInput")
    ft2 = nc.dram_tensor("ft2", (PT, NCH, QPC * CT_SZ[2]), FP8, kind="ExternalInput")
    auxh = nc.dram_tensor("auxh", (NH, 2200), BF16, kind="ExternalInput")
    auxj = nc.dram_tensor("auxj", (128, 2, NB), BF16, kind="ExternalInput")
    fbd = nc.dram_tensor("fbd", (3, 128, BL, TO), BF16, kind="ExternalInput")
    spnd = nc.dram_tensor("spnd", (3, 128, BL, TO), BF16, kind="ExternalInput")
    tfd = nc.dram_tensor("tfd", (128, 3, K), F32, kind="ExternalInput")
    ident = nc.dram_tensor("ident", (128, 128), BF16, kind="ExternalInput")
    part = nc.dram_tensor("part", (128, 2, NU), F32, kind="ExternalOutput")
    ftd = [ft0, ft1, ft2]

    es = ExitStack()
    with es:
        jt_sb = es.enter_context(nc.sbuf_tensor("jt_sb", [PT, NCH, QPC, BL, NJP], FP8))
        ft_sb = es.enter_context(nc.sbuf_tensor("ft_sb", [PT, NCH, 2048], FP8))
        dg_sb = es.enter_context(nc.sbuf_tensor("dg_sb", [128, 2, NPAIR, 2, 128], FP8))
        ident_sb = es.enter_context(nc.sbuf_tensor("ident_sb", [128, 128], BF16))
        tf_sb = es.enter_context(nc.sbuf_tensor("tf_sb", [128, 3, K], F32))
        auxh_sb = es.enter_context(nc.sbuf_tensor("auxh_sb", [NH, 2200], BF16))
        auxj_sb = es.enter_context(nc.sbuf_tensor("auxj_sb", [128, 2, NB], BF16))
        spatj_sb = es.enter_context(nc.sbuf_tensor("spatj_sb", [128, 2, 128], BF16))
        up_sb = es.enter_context(nc.sbuf_tensor("up_sb", [128, 2, BL, 2, 752], FP8))
        fb_sb = es.enter_context(nc.sbuf_tensor("fb_sb", [128, 2, BL, TO], BF16))
        spn_sb = es.enter_context(nc.sbuf_tensor("spn_sb", [128, 2, BL, TO], BF16))
        tmpa_sb = es.enter_context(nc.sbuf_tensor("tmpa_sb", [128, 2, TO], F32))
        tmpb_sb = es.enter_context(nc.sbuf_tensor("tmpb_sb", [128, 2, TO], F32))
        junk_sb = es.enter_context(nc.sbuf_tensor("junk_sb", [128, 2, TO], F32))
        rr_sb = es.enter_context(nc.sbuf_tensor("rr_sb", [128, 2, NU], F32))
        spat_ps = es.enter_context(nc.psum_tensor("spat_ps", [128, 128], F32))
        up_ps = es.enter_context(nc.psum_tensor("up_ps", [128, BL, 2, 512], F32))
        acc_ps = es.enter_context(nc.psum_tensor("acc_ps", [128, 3, 512], F32))
        jsems = [es.enter_context(nc.semaphore(f"jsem{i}")) for i in range(NPC)]
        fsems = [es.enter_context(nc.semaphore(f"fsem{i}")) for i in range(NPC)]
        psem = es.enter_context(nc.semaphore("psem"))    # spatial tile done
        scsem = es.enter_context(nc.semaphore("scsem"))  # spat copied to sbuf
        upsem = es.enter_context(nc.semaphore("upsem"))  # upsample done per (ct,b)
        ucsem = es.enter_context(nc.semaphore("ucsem"))  # up copied per (ct,b)
        dgsem = [es.enter_context(nc.semaphore(f"dgsem{i}")) for i in range(3)]
        chsem = es.enter_context(nc.semaphore("chsem"))  # conv unit done
        gsem = es.enter_context(nc.semaphore("gsem"))    # tmpb built (acc free)
        tsem = es.enter_context(nc.semaphore("tsem"))    # tmpa ready
        asem = es.enter_context(nc.semaphore("asem"))    # act accums done
        rsem = es.enter_context(nc.semaphore("rsem"))    # res ready
        osem = es.enter_context(nc.semaphore("osem"))
        osem2 = es.enter_context(nc.semaphore("osem2"))
        dsem = es.enter_context(nc.semaphore("dsem"))    # ident dma
        hsem = es.enter_context(nc.semaphore("hsem"))    # tfd dma
        asem2 = es.enter_context(nc.semaphore("asem2"))  # aux dma
        fbsems = [es.enter_context(nc.semaphore(f"fbsem{i}")) for i in range(3)]
        block = es.enter_context(nc.Block())

        jt_h = jt_sb[0:PT, 0, 0, 0, 0:1]
        ft_h = ft_sb[0:PT, 0, 0:1]

        def jt_ap(ch, q):
            return APc(jt_h.tensor, ch * 2048 + q * (BL * NJP),
                       [[ROW_JT, PT], [BL * NJP, 2], [NJP, BL], [1, NJP]])

        def ft_ap(ch, q, mc):
            return APc(ft_h.tensor, ch * 2048 + q * mc,
                       [[ROW_FT, PT], [mc, 2], [1, mc]])

        @block.sync
        def _(sync):
            sync.dma_start(ident_sb[:], ident[:]).then_inc(dsem, 16)
            # lead-in: interleave jt and ft0 pieces; tfd after pair 0 (gates
            # diag builds ~14us), aux after last pair (gates upsample ~31us)
            for i in range(NPC):
                sync.dma_start(jt_sb[:, CPP * i:CPP * (i + 1)],
                               jtd[:, CPP * i:CPP * (i + 1)]).then_inc(jsems[i], 16)
                sync.dma_start(
                    ft_sb[:, CPP * i:CPP * (i + 1), 0:QPC * CT_SZ[0]],
                    ftd[0][:, CPP * i:CPP * (i + 1)]).then_inc(fsems[i], 16)
                if i == 0:
                    sync.dma_start(tf_sb[:], tfd[:]).then_inc(hsem, 16)
            sync.dma_start(auxh_sb[:], auxh[:]).then_inc(asem2, 16)
            sync.dma_start(auxj_sb[:], auxj[:]).then_inc(dsem, 16)
            sync.dma_start(fb_sb[:, 0], fbd[0]).then_inc(fbsems[0], 16)
            sync.wait_ge(fbsems[0], 16)
            sync.dma_start(spn_sb[:, 0], spnd[0]).then_inc(fbsems[0], 16)
            # ft1 after spatial ct0 released ft_sb
            sync.wait_ge(psem, 1)
            for i in range(NPC):
                sync.wait_ge(fsems[i], 16)
                sync.dma_start(
                    ft_sb[:, CPP * i:CPP * (i + 1), 0:QPC * CT_SZ[1]],
                    ftd[1][:, CPP * i:CPP * (i + 1)]).then_inc(fsems[i], 16)
            sync.dma_start(fb_sb[:, 1], fbd[1]).then_inc(fbsems[1], 16)
            sync.wait_ge(fbsems[1], 16)
            sync.dma_start(spn_sb[:, 1], spnd[1]).then_inc(fbsems[1], 16)
            sync.wait_ge(psem, 2)
            for i in range(NPC):
                sync.wait_ge(fsems[i], 32)
                sync.dma_start(
                    ft_sb[:, CPP * i:CPP * (i + 1), 0:QPC * CT_SZ[2]],
                    ftd[2][:, CPP * i:CPP * (i + 1)]).then_inc(fsems[i], 16)
            # fb slot 0 reused for ct2: wait units 0,1 postproc done
            sync.wait_ge(gsem, 2)
            sync.dma_start(fb_sb[:, 0], fbd[2]).then_inc(fbsems[2], 16)
            sync.wait_ge(fbsems[2], 16)
            sync.dma_start(spn_sb[:, 0], spnd[2]).then_inc(fbsems[2], 16)
            # output: single staging DMA; host reassembles
            sync.wait_ge(rsem, NU)
            sync.wait_ge(asem, NU)
            sync.dma_start(part[:], rr_sb[:]).then_inc(osem, 16)
            sync.wait_ge(osem, 16)

        @block.tensor
        def _(tensor):
            def spatial(ct):
                mc = CT_SZ[ct]
                if ct >= 1:
                    tensor.wait_ge(scsem, ct)   # spat_ps drained by copy
                mm = None
                for pc in range(NPC):
                    if ct == 0 and pc == NPC - 1:
                        # keep PE busy ~3us so last burst + upsample run at
                        # full pstate (ramp needs >3us continuous execution)
                        for _ in range(30):
                            tensor.matmul(acc_ps[0:128, 0, 0:128],
                                          ident_sb[0:128, 0:128],
                                          ident_sb[0:128, 0:128],
                                          start=True, stop=True,
                                          skip_group_check=True)
                    tensor.wait_ge(fsems[pc], 16 * (ct + 1))
                    if ct == 0:
                        tensor.wait_ge(jsems[pc], 16)
                    for ch in range(CPP * pc, CPP * (pc + 1)):
                        for q in range(0, QPC, 2):
                            mm = tensor.matmul(
                                spat_ps[0:128, 0:mc],
                                jt_ap(ch, q), ft_ap(ch, q, mc),
                                start=(ch == 0 and q == 0),
                                stop=(ch == NCH - 1 and q == QPC - 2),
                                skip_group_check=True, perf_mode=DR)
                mm.then_inc(psem, 1)

            def upsample(ct):
                mc, off = CT_SZ[ct], CT_OFF[ct]
                if ct == 0:
                    tensor.wait_ge(asem2, 16)
                    tensor.wait_ge(dsem, 32)
                tensor.wait_ge(scsem, ct + 1)
                if ct >= 1:
                    tensor.wait_ge(ucsem, 2 * ct)   # up_ps drained by copies
                for b in range(BL):
                    for h in range(2):
                        tensor.matmul(up_ps[0:mc, b, h, 0:375],
                                      auxh_sb[0:NH, 350 * b + off:350 * b + off + mc],
                                      auxh_sb[0:NH, 700 + 750 * b + 375 * h:
                                              700 + 750 * b + 375 * (h + 1)],
                                      start=True, stop=False,
                                      skip_group_check=True)
                        tensor.matmul(
                            up_ps[0:mc, b, h, 0:375],
                            spatj_sb[64 * b:64 * b + 60, ct % 2, 0:mc],
                            auxj_sb[64 * b:64 * b + 60, b, 375 * h:375 * (h + 1)],
                            start=False, stop=True,
                            skip_group_check=True).then_inc(upsem, 1)

            def conv(u):
                ct, b = u // 2, u % 2
                mc = CT_SZ[ct]
                lane = u % 3
                tensor.wait_ge(dgsem[ct], 2 * NPAIR)
                tensor.wait_ge(ucsem, 2 * ct + b + 1)
                if u >= 3:
                    tensor.wait_ge(gsem, u - 2)
                mm = None
                for p in range(NPAIR):
                    mm = tensor.matmul(
                        acc_ps[0:mc, lane, 0:TO],
                        dg_sb[0:mc, DG_SLOT[ct], p, 0:2, 0:mc],
                        up_sb[0:mc, ct % 2, b, 0:2, 2 * p:2 * p + TO],
                        start=(p == 0), stop=(p == NPAIR - 1),
                        skip_group_check=True, perf_mode=DR)
                mm.then_inc(chsem, 1)

            spatial(0)
            upsample(0)
            conv(0)
            spatial(1)
            upsample(1)
            conv(1)
            spatial(2)
            upsample(2)
            conv(2)
            conv(3)
            conv(4)
            conv(5)

        def dg_build(ct, p0, p1, ts_fn, sem_target):
            mc = CT_SZ[ct]
            for p in range(p0, p1):
                for j in range(2):
                    col = 2 * p + j
                    ts_fn(dg_sb[0:mc, DG_SLOT[ct], p, j, 0:mc],
                          ident_sb[0:mc, 0:mc],
                          tf_sb[0:mc, ct, col:col + 1]).then_inc(sem_target[ct], 1)

        @block.gpsimd
        def _(gp):
            gp.wait_ge(dsem, 16)
            gp.wait_ge(hsem, 16)
            d0, p0, a0 = DG_SPLIT[0]
            dg_build(0, d0, d0 + p0, gp.tensor_scalar_mul, dgsem)
            d1, p1, a1 = DG_SPLIT[1]
            dg_build(1, d1, d1 + p1, gp.tensor_scalar_mul, dgsem)
            gp.wait_ge(chsem, 2)   # slot 0 free after conv units 0,1
            d2, p2, a2 = DG_SPLIT[2]
            dg_build(2, d2, d2 + p2, gp.tensor_scalar_mul, dgsem)


        @block.vector
        def _(vector):
            vector.wait_ge(dsem, 16)
            vector.wait_ge(hsem, 16)
            d0, p0, a0 = DG_SPLIT[0]
            dg_build(0, 0, d0, vector.tensor_scalar_mul, dgsem)
            d1, p1, a1 = DG_SPLIT[1]
            dg_build(1, 0, 40, vector.tensor_scalar_mul, dgsem)

            def postproc(u):
                ct, b = u // 2, u % 2
                mc = CT_SZ[ct]
                lane, us = u % 3, u % 2
                fslot = [0, 1, 0][ct]
                vector.wait_ge(chsem, u + 1)
                vector.wait_ge(fbsems[ct], 32)
                if u >= 2:
                    vector.wait_ge(asem, u - 1)   # tmpa slot us free
                vector.scalar_tensor_tensor(
                    tmpa_sb[0:mc, us, :], acc_ps[0:mc, lane, 0:TO],
                    1.0 / TSC, fb_sb[0:mc, fslot, b, :], MU, AD).then_inc(tsem, 1)
                vector.scalar_tensor_tensor(
                    tmpb_sb[0:mc, us, :], acc_ps[0:mc, lane, 0:TO],
                    1.0 / TSC, spn_sb[0:mc, fslot, b, :], MU, MU).then_inc(gsem, 1)
                vector.tensor_reduce(rr_sb[0:mc, 1, u:u + 1],
                                     tmpb_sb[0:mc, us, :],
                                     AXX, AD).then_inc(rsem, 1)

            dg_build(1, 40, d1, vector.tensor_scalar_mul, dgsem)
            postproc(0)
            d2, p2, a2 = DG_SPLIT[2]
            vector.wait_ge(chsem, 2)   # slot 0 free
            postproc(1)
            dg_build(2, 0, 40, vector.tensor_scalar_mul, dgsem)
            postproc(2)
            dg_build(2, 40, d2, vector.tensor_scalar_mul, dgsem)
            postproc(3)
            postproc(4)
            postproc(5)

        @block.scalar
        def _(scalar):
            scalar.wait_ge(dsem, 16)
            scalar.wait_ge(hsem, 16)
            d0, p0, a0 = DG_SPLIT[0]
            if a0:
                dg_build(0, d0 + p0, NPAIR,
                         lambda o, i, s: scalar.mul(o, i, s), dgsem)

            def spat_copy(ct):
                scalar.wait_ge(psem, ct + 1)
                if ct >= 2:
                    scalar.wait_ge(upsem, 2 * (ct - 1))  # spatj slot free
                scalar.mul(spatj_sb[0:128, ct % 2, 0:CT_SZ[ct]],
                           spat_ps[0:128, 0:CT_SZ[ct]],
                           1.0 / FSC).then_inc(scsem, 1)

            def up_copy(ct, b):
                mc = CT_SZ[ct]
                us = ct % 2
                scalar.wait_ge(upsem, 4 * ct + 2 * b + 1)
                if ct >= 2 and b == 0:
                    scalar.wait_ge(chsem, 2)  # up_sb slot0 free after units 0,1
                scalar.activation(up_sb[0:mc, us, b, 0, 0:375],
                                  up_ps[0:mc, b, 0, 0:375], CPY)
                scalar.activation(up_sb[0:mc, us, b, 1, 0:374],
                                  up_ps[0:mc, b, 0, 1:375], CPY)
                scalar.wait_ge(upsem, 4 * ct + 2 * b + 2)
                scalar.activation(up_sb[0:mc, us, b, 0, 375:750],
                                  up_ps[0:mc, b, 1, 0:375], CPY)
                scalar.activation(up_sb[0:mc, us, b, 1, 374:749],
                                  up_ps[0:mc, b, 1, 0:375],
                                  CPY).then_inc(ucsem, 1)

            def accums(u):
                ct, b = u // 2, u % 2
                mc = CT_SZ[ct]
                us = u % 2
                scalar.wait_ge(tsem, u + 1)
                scalar.activation(junk_sb[0:mc, us, :], tmpa_sb[0:mc, us, :],
                                  EXP,
                                  accum_out=rr_sb[0:mc, 0, u:u + 1]).then_inc(asem, 1)

            spat_copy(0)
            up_copy(0, 0)
            up_copy(0, 1)
            accums(0)
            spat_copy(1)
            up_copy(1, 0)
            up_copy(1, 1)
            spat_copy(2)
            up_copy(2, 0)
            up_copy(2, 1)
            accums(1)
            accums(2)
            accums(3)
            accums(4)
            accums(5)
    return nc


_NC_CACHE = {}


def _host_prep(inputs):
    img = np.asarray(inputs["batched_image"], dtype=np.float32)
    spikes = np.asarray(inputs["batched_spikes"], dtype=np.float32)
    em = np.asarray(inputs["eye_movements"]).astype(np.int64)
    tmask = np.asarray(inputs["time_mask"], dtype=np.float32)
    sel = np.asarray(inputs["forward_sel"]).astype(np.int64)
    fw = np.asarray(inputs["forward_weights"], dtype=np.float32)
    F = np.asarray(inputs["stacked_flat_spat_filters"], dtype=np.float32)
    tcf = np.asarray(inputs["stacked_timecourse_filters"], dtype=np.float32)
    fbg = np.asarray(inputs["precomputed_feedback_gensig"], dtype=np.float32)
    histf = np.asarray(inputs["precomputed_history_frames"], dtype=np.float32)

    # jitter on host (pure gather, exact)
    jit = np.zeros((B, NJ, H, W), dtype=np.float32)
    for b in range(B):
        for f in range(NJ):
            dy, dx = int(em[b, f, 0]), int(em[b, f, 1])
            ys, xs = max(0, -dy), max(0, -dx)
            ye, xe = min(H, H - dy), min(W, W - dx)
            if ye > ys and xe > xs:
                jit[b, f, ys:ye, xs:xe] = img[b, ys + dy:ye + dy, xs + dx:xe + dx]
    jitT = jit.reshape(B, NJ, P).transpose(0, 2, 1)                 # (B,P,NJ)
    # pixel index = ch*2048 + q*128 + p
    jitp = np.zeros((B, NCH, PT, QPC, NJP), dtype=np.float32)
    jitp[..., 0:NJ] = jitT.reshape(B, NCH, QPC, PT, NJ).transpose(0, 1, 3, 2, 4)
    jitc = jitp.astype(F8)
    # jtd per batch group: (PT, NCH, QPC, BL, NJP)
    jt_h = [np.ascontiguousarray(
        np.stack([jitc[BL * bg + b] for b in range(BL)], axis=3)
        .transpose(1, 0, 2, 3, 4))
        for bg in range(GB)]

    FTf = F.T * np.float32(FSC)                                     # (P,C)
    ftc_h = {}
    for cg in range(GC):
        for ct in range(3):
            mc, off = CT_SZ[ct], CT_OFF[ct]
            X = FTf[:, cg * CL + off:cg * CL + off + mc]
            ftc_h[(cg, ct)] = np.ascontiguousarray(
                X.reshape(NCH, QPC, PT, mc).transpose(2, 0, 1, 3)
                .reshape(PT, NCH, QPC * mc)).astype(F8)

    # upsample mixing matrix M[f,t]
    Mm = np.zeros((B, NF, NB), dtype=np.float32)
    tix = np.arange(NB)
    for b in range(B):
        np.add.at(Mm[b], (sel[b, :, 0], tix), fw[b, :, 0])
        np.add.at(Mm[b], (sel[b, :, 1], tix), fw[b, :, 1])
    Mmb = Mm.astype(BF)

    mv = tmask * np.float32(MAGIC)                                  # (B,500)
    with np.errstate(divide="ignore"):
        lmv = np.log(mv).astype(np.float32)
    spn_all = -(spikes[:, :, K:] * mv[:, None, :])                  # (B,C,500)
    fb5 = fbg[:, :, :TO]
    fbl_all = fb5 + lmv[:, None, :]                                 # fb + log(mv)
    # host part of linear term: sum_{c,t} spn*fb per batch
    hconst = np.einsum('bct,bct->b', spn_all.astype(np.float64), fb5.astype(np.float64))
    histb = histf.astype(BF)                                        # (B,NH,C)
    identity = np.eye(128, dtype=np.float32).astype(BF)
    tcf_s = tcf * np.float32(TSC)

    in_maps = []
    for i in range(8):
        bg, cg = i // GC, i % GC
        bs = slice(BL * bg, BL * (bg + 1))
        cs = slice(CL * cg, CL * (cg + 1))
        fbp = np.zeros((3, 128, BL, TO), dtype=BF)
        spnp = np.zeros((3, 128, BL, TO), dtype=BF)
        for ct in range(3):
            mc, off = CT_SZ[ct], CT_OFF[ct]
            for b in range(BL):
                fbp[ct, 0:mc, b, :] = fbl_all[BL * bg + b,
                                              cg * CL + off:cg * CL + off + mc, :].astype(BF)
                spnp[ct, 0:mc, b, :] = spn_all[BL * bg + b,
                                               cg * CL + off:cg * CL + off + mc, :].astype(BF)
        auxhp = np.zeros((NH, 2200), dtype=BF)
        auxhp[:, 0:700] = histb[bs][:, :, cs].transpose(1, 0, 2).reshape(NH, 700)
        auxhp[:, 700:2200] = Mmb[bs][:, 0:NH].transpose(1, 0, 2).reshape(NH, 1500)
        auxjp = np.zeros((128, 2, NB), dtype=BF)
        auxjp[0:NJ] = Mmb[bs][:, NH:NF].transpose(1, 0, 2)
        auxjp[64:64 + NJ] = auxjp[0:NJ]
        tfp = np.zeros((128, 3, K), dtype=np.float32)
        for ct in range(3):
            mc, off = CT_SZ[ct], CT_OFF[ct]
            tfp[0:mc, ct] = tcf_s[cg * CL + off:cg * CL + off + mc]
        in_maps.append({
            "jtd": jt_h[bg],
            "ft0": ftc_h[(cg, 0)],
            "ft1": ftc_h[(cg, 1)],
            "ft2": ftc_h[(cg, 2)],
            "auxh": auxhp,
            "auxj": auxjp,
            "fbd": fbp,
            "spnd": spnp,
            "tfd": tfp,
            "ident": identity,
        })
    return in_maps, hconst


def kernel(**inputs) -> np.ndarray:
    in_maps, hconst = _host_prep(inputs)
    if "nc" not in _NC_CACHE:
        _NC_CACHE["nc"] = _build_nc()
    nc = _NC_CACHE["nc"]

    if os.environ.get("KTRACE"):
        res = run_bass_kernel_spmd(
            nc, in_maps, core_ids=list(range(8)), trace=True,
            trace_cores=[0], tmpdir=os.environ.get("KTRACE_DIR") or None)
        kernel.last_results = res
    else:
        res = run_bass_kernel_spmd(nc, in_maps, core_ids=list(range(8)))
    out = np.array(hconst, dtype=np.float64)
    for i in range(8):
        bg = i // GC
        pr = res.results[i]["part"]
        for u in range(NU):
            ct, b = u // 2, u % 2
            mc = CT_SZ[ct]
            out[BL * bg + b] += pr[0:mc, :, u].sum(dtype=np.float64)
    return out.astype(np.float32)


# revision 7
# speedup vs baseline: 1.2709x; 1.0176x over previous
import os
import numpy as np
import ml_dtypes
from contextlib import ExitStack
import concourse.bass as bass
import concourse.mybir as mybir
from concourse.ap import AP as APc
from concourse.bass_utils import run_bass_kernel_spmd

B, H, W = 8, 160, 256
C, K = 700, 250
NB = 750
NH, NJ = 30, 60
NJP = 64
NF = 90
TO = 500
P = H * W
MAGIC = 400.0 / 750.0

GB, GC = 4, 2      # batch groups x cell groups
BL = B // GB       # 2 batches per core
CL = C // GC       # 350 cells per core
PT = 128
NPT = P // PT      # 320 pixel tiles
QPC = 16           # pixel tiles per chunk
NCH = NPT // QPC   # 20 chunks
NPC = 6            # DMA pieces per (jt|ft) stream
SPANS = [(0, 4), (4, 4), (8, 4), (12, 4), (16, 3), (19, 1)]
CT_SZ = [94, 128, 128]   # smallest tile first: shortest DMA lead-in
CT_OFF = [0, 94, 222]
NPAIR = K // 2     # 125 tap pairs
NU = 6             # conv units = 3 tiles x 2 batches
FSC = 64.0
TSC = 8.0
F32 = mybir.dt.float32
BF16 = mybir.dt.bfloat16
FP8 = mybir.dt.float8e4
BF = ml_dtypes.bfloat16
F8 = ml_dtypes.float8_e4m3fn
DR = mybir.MatmulPerfMode.DoubleRow

ROW_JT = NCH * QPC * BL * NJP    # 40960 per-partition elems
ROW_FT = NCH * 2048              # ft_sb free width
# diag build split per set: (dve_pairs, pool_pairs, act_pairs)
DG_SPLIT = [(65, 30, 30), (65, 60, 0), (80, 45, 0)]
DG_SLOT = [0, 1, 0]


def _build_nc():
    CPY = mybir.ActivationFunctionType.Copy
    EXP = mybir.ActivationFunctionType.Exp
    MU = mybir.AluOpType.mult
    AD = mybir.AluOpType.add
    AXX = mybir.AxisListType.X
    nc = bass.Bass()
    jtd = nc.dram_tensor("jtd", (PT, NCH, QPC, BL, NJP), FP8, kind="ExternalInput")
    ft0 = nc.dram_tensor("ft0", (PT, NCH, QPC * CT_SZ[0]), FP8, kind="ExternalInput")
    ft1 = nc.dram_tensor("ft1", (PT, NCH, QPC * CT_SZ[1]), FP8, kind="ExternalInput")
    ft2 = nc.dram_tensor("ft2", (PT, NCH, QPC * CT_SZ[2]), FP8, kind="ExternalInput")
    auxh = nc.dram_tensor("auxh", (NH, 2200), BF16, kind="ExternalInput")
    auxj = nc.dram_tensor("auxj", (128, 2, NB), BF16, kind="ExternalInput")
    fbd = nc.dram_tensor("fbd", (3, 128, BL, TO), BF16, kind="ExternalInput")
    spnd = nc.dram_tensor("spnd", (3, 128, BL, TO), BF16, kind="ExternalInput")
    tfd = nc.dram_tensor("tfd", (128, 3, K), F32, kind="ExternalInput")
    ident = nc.dram_tensor("ident", (128, 128), BF16, kind="ExternalInput")
    part = nc.dram_tensor("part", (128, NU, 2), F32, kind="ExternalOutput")
    ftd = [ft0, ft1, ft2]

    es = ExitStack()
    with es:
        jt_sb = es.enter_context(nc.sbuf_tensor("jt_sb", [PT, NCH, QPC, BL, NJP], FP8))
        ft_sb = es.enter_context(nc.sbuf_tensor("ft_sb", [PT, NCH, 2048], FP8))
        dg_sb = es.enter_context(nc.sbuf_tensor("dg_sb", [128, 2, NPAIR, 2, 128], FP8))
        ident_sb = es.enter_context(nc.sbuf_tensor("ident_sb", [128, 128], BF16))
        tf_sb = es.enter_context(nc.sbuf_tensor("tf_sb", [128, 3, K], F32))
        auxh_sb = es.enter_context(nc.sbuf_tensor("auxh_sb", [NH, 2200], BF16))
        auxj_sb = es.enter_context(nc.sbuf_tensor("auxj_sb", [128, 2, NB], BF16))
        spatj_sb = es.enter_context(nc.sbuf_tensor("spatj_sb", [128, 2, 128], BF16))
        up_sb = es.enter_context(nc.sbuf_tensor("up_sb", [128, 2, BL, 2, 752], FP8))
        fb_sb = es.enter_context(nc.sbuf_tensor("fb_sb", [128, 2, BL, TO], BF16))
        spn_sb = es.enter_context(nc.sbuf_tensor("spn_sb", [128, 2, BL, TO], BF16))
        tmpa_sb = es.enter_context(nc.sbuf_tensor("tmpa_sb", [128, 2, TO], F32))
        tmpb_sb = es.enter_context(nc.sbuf_tensor("tmpb_sb", [128, 2, TO], F32))
        junk_sb = es.enter_context(nc.sbuf_tensor("junk_sb", [128, 2, TO], F32))
        rr_sb = es.enter_context(nc.sbuf_tensor("rr_sb", [128, NU, 2], F32))
        spat_ps = es.enter_context(nc.psum_tensor("spat_ps", [128, 128], F32))
        up_ps = es.enter_context(nc.psum_tensor("up_ps", [128, BL, 2, 512], F32))
        acc_ps = es.enter_context(nc.psum_tensor("acc_ps", [128, 3, 512], F32))
        jsems = [es.enter_context(nc.semaphore(f"jsem{i}")) for i in range(NPC)]
        fsems = [es.enter_context(nc.semaphore(f"fsem{i}")) for i in range(NPC)]
        psem = es.enter_context(nc.semaphore("psem"))    # spatial tile done
        scsem = es.enter_context(nc.semaphore("scsem"))  # spat copied to sbuf
        upsem = es.enter_context(nc.semaphore("upsem"))  # upsample done per (ct,b)
        ucsem = es.enter_context(nc.semaphore("ucsem"))  # up copied per (ct,b)
        dgsem = [es.enter_context(nc.semaphore(f"dgsem{i}")) for i in range(3)]
        chsem = es.enter_context(nc.semaphore("chsem"))  # conv unit done
        gsem = es.enter_context(nc.semaphore("gsem"))    # tmpb built (acc free)
        tsem = es.enter_context(nc.semaphore("tsem"))    # tmpa ready
        asem = es.enter_context(nc.semaphore("asem"))    # act accums done
        rsem = es.enter_context(nc.semaphore("rsem"))    # res ready
        osem = es.enter_context(nc.semaphore("osem"))
        osem2 = es.enter_context(nc.semaphore("osem2"))
        dsem = es.enter_context(nc.semaphore("dsem"))    # ident dma
        hsem = es.enter_context(nc.semaphore("hsem"))    # tfd dma
        asem2 = es.enter_context(nc.semaphore("asem2"))  # auxh dma
        ajs = [es.enter_context(nc.semaphore(f"ajs{i}")) for i in range(2)]
        fbsems = [es.enter_context(nc.semaphore(f"fbsem{i}")) for i in range(3)]
        block = es.enter_context(nc.Block())

        jt_h = jt_sb[0:PT, 0, 0, 0, 0:1]
        ft_h = ft_sb[0:PT, 0, 0:1]

        def jt_ap(ch, q):
            return APc(jt_h.tensor, ch * 2048 + q * (BL * NJP),
                       [[ROW_JT, PT], [BL * NJP, 2], [NJP, BL], [1, NJP]])

        def ft_ap(ch, q, mc):
            return APc(ft_h.tensor, ch * 2048 + q * mc,
                       [[ROW_FT, PT], [mc, 2], [1, mc]])

        @block.sync
        def _(sync):
            sync.dma_start(ident_sb[:], ident[:]).then_inc(dsem, 16)
            # lead-in: interleave jt and ft0 pieces; tfd after pair 0 (gates
            # diag builds ~14us), aux after last pair (gates upsample ~31us)
            for i, (c0, cn) in enumerate(SPANS):
                sync.dma_start(jt_sb[:, c0:c0 + cn],
                               jtd[:, c0:c0 + cn]).then_inc(jsems[i], 16)
                sync.dma_start(
                    ft_sb[:, c0:c0 + cn, 0:QPC * CT_SZ[0]],
                    ftd[0][:, c0:c0 + cn]).then_inc(fsems[i], 16)
                if i == 0:
                    sync.dma_start(tf_sb[:], tfd[:]).then_inc(hsem, 16)
            sync.dma_start(auxh_sb[:], auxh[:]).then_inc(asem2, 16)
            sync.dma_start(auxj_sb[:, 0:1], auxj[:, 0:1]).then_inc(ajs[0], 16)
            sync.dma_start(auxj_sb[:, 1:2], auxj[:, 1:2]).then_inc(ajs[1], 16)
            sync.dma_start(fb_sb[:, 0], fbd[0]).then_inc(fbsems[0], 16)
            sync.wait_ge(fbsems[0], 16)
            sync.dma_start(spn_sb[:, 0], spnd[0]).then_inc(fbsems[0], 16)
            # ft1 after spatial ct0 released ft_sb
            sync.wait_ge(psem, 1)
            for i, (c0, cn) in enumerate(SPANS):
                sync.wait_ge(fsems[i], 16)
                sync.dma_start(
                    ft_sb[:, c0:c0 + cn, 0:QPC * CT_SZ[1]],
                    ftd[1][:, c0:c0 + cn]).then_inc(fsems[i], 16)
            sync.dma_start(fb_sb[:, 1], fbd[1]).then_inc(fbsems[1], 16)
            sync.wait_ge(fbsems[1], 16)
            sync.dma_start(spn_sb[:, 1], spnd[1]).then_inc(fbsems[1], 16)
            sync.wait_ge(psem, 2)
            for i, (c0, cn) in enumerate(SPANS):
                sync.wait_ge(fsems[i], 32)
                sync.dma_start(
                    ft_sb[:, c0:c0 + cn, 0:QPC * CT_SZ[2]],
                    ftd[2][:, c0:c0 + cn]).then_inc(fsems[i], 16)
            # fb slot 0 reused for ct2: wait units 0,1 postproc done
            sync.wait_ge(gsem, 2)
            sync.dma_start(fb_sb[:, 0], fbd[2]).then_inc(fbsems[2], 16)
            sync.wait_ge(fbsems[2], 16)
            sync.dma_start(spn_sb[:, 0], spnd[2]).then_inc(fbsems[2], 16)
            # output: staging DMAs; host reassembles. First covers units
            # 0-4 (overlapped under conv5), last ships only unit 5's columns.
            sync.wait_ge(rsem, NU - 1)
            sync.wait_ge(asem, NU - 1)
            sync.dma_start(part[:, 0:NU - 1],
                           rr_sb[0:128, 0:NU - 1]).then_inc(osem, 16)
            sync.wait_ge(rsem, NU)
            sync.wait_ge(asem, NU)
            sync.dma_start(part[:, NU - 1:NU],
                           rr_sb[0:128, NU - 1:NU]).then_inc(osem2, 16)
            sync.wait_ge(osem, 16)
            sync.wait_ge(osem2, 16)

        @block.tensor
        def _(tensor):
            def spatial(ct):
                mc = CT_SZ[ct]
                if ct >= 1:
                    tensor.wait_ge(scsem, ct)   # spat_ps drained by copy
                mm = None
                for pc, (c0, cn) in enumerate(SPANS):
                    if ct == 0 and pc == 4:
                        # keep PE busy ~3us so last bursts + upsample run at
                        # full pstate (ramp needs >3us continuous execution)
                        for _ in range(30):
                            tensor.matmul(acc_ps[0:128, 0, 0:128],
                                          ident_sb[0:128, 0:128],
                                          ident_sb[0:128, 0:128],
                                          start=True, stop=True,
                                          skip_group_check=True)
                    tensor.wait_ge(fsems[pc], 16 * (ct + 1))
                    if ct == 0:
                        tensor.wait_ge(jsems[pc], 16)
                    for ch in range(c0, c0 + cn):
                        for q in range(0, QPC, 2):
                            mm = tensor.matmul(
                                spat_ps[0:128, 0:mc],
                                jt_ap(ch, q), ft_ap(ch, q, mc),
                                start=(ch == 0 and q == 0),
                                stop=(ch == NCH - 1 and q == QPC - 2),
                                skip_group_check=True, perf_mode=DR)
                mm.then_inc(psem, 1)

            def upsample(ct):
                mc, off = CT_SZ[ct], CT_OFF[ct]
                if ct == 0:
                    tensor.wait_ge(asem2, 16)
                tensor.wait_ge(scsem, ct + 1)
                if ct >= 1:
                    tensor.wait_ge(ucsem, 2 * ct)   # up_ps drained by copies
                for b in range(BL):
                    if ct == 0:
                        tensor.wait_ge(ajs[b], 16)
                    for h in range(2):
                        tensor.matmul(up_ps[0:mc, b, h, 0:375],
                                      auxh_sb[0:NH, 350 * b + off:350 * b + off + mc],
                                      auxh_sb[0:NH, 700 + 750 * b + 375 * h:
                                              700 + 750 * b + 375 * (h + 1)],
                                      start=True, stop=False,
                                      skip_group_check=True)
                        tensor.matmul(
                            up_ps[0:mc, b, h, 0:375],
                            spatj_sb[64 * b:64 * b + 60, ct % 2, 0:mc],
                            auxj_sb[64 * b:64 * b + 60, b, 375 * h:375 * (h + 1)],
                            start=False, stop=True,
                            skip_group_check=True).then_inc(upsem, 1)

            def conv(u, p0=0, p1=NPAIR):
                ct, b = u // 2, u % 2
                mc = CT_SZ[ct]
                lane = u % 3
                if p0 == 0:
                    tensor.wait_ge(dgsem[ct], 2 * NPAIR)
                    tensor.wait_ge(ucsem, 2 * ct + b + 1)
                    if u >= 3:
                        tensor.wait_ge(gsem, u - 2)
                mm = None
                for p in range(p0, p1):
                    mm = tensor.matmul(
                        acc_ps[0:mc, lane, 0:TO],
                        dg_sb[0:mc, DG_SLOT[ct], p, 0:2, 0:mc],
                        up_sb[0:mc, ct % 2, b, 0:2, 2 * p:2 * p + TO],
                        start=(p == 0), stop=(p == NPAIR - 1),
                        skip_group_check=True, perf_mode=DR)
                if p1 == NPAIR:
                    mm.then_inc(chsem, 1)

            spatial(0)
            upsample(0)
            conv(0)
            spatial(1)
            conv(1, 0, 62)
            upsample(1)
            conv(1, 62, NPAIR)
            conv(2)
            spatial(2)
            conv(3, 0, 62)
            upsample(2)
            conv(3, 62, NPAIR)
            conv(4)
            conv(5)

        def dg_build(ct, p0, p1, ts_fn, sem_target):
            mc = CT_SZ[ct]
            for p in range(p0, p1):
                for j in range(2):
                    col = 2 * p + j
                    ts_fn(dg_sb[0:mc, DG_SLOT[ct], p, j, 0:mc],
                          ident_sb[0:mc, 0:mc],
                          tf_sb[0:mc, ct, col:col + 1]).then_inc(sem_target[ct], 1)

        @block.gpsimd
        def _(gp):
            gp.wait_ge(dsem, 16)
            gp.wait_ge(hsem, 16)
            d0, p0, a0 = DG_SPLIT[0]
            dg_build(0, d0, d0 + p0, gp.tensor_scalar_mul, dgsem)
            d1, p1, a1 = DG_SPLIT[1]
            dg_build(1, d1, d1 + p1, gp.tensor_scalar_mul, dgsem)
            gp.wait_ge(chsem, 2)   # slot 0 free after conv units 0,1
            d2, p2, a2 = DG_SPLIT[2]
            dg_build(2, d2, d2 + p2, gp.tensor_scalar_mul, dgsem)


        @block.vector
        def _(vector):
            vector.wait_ge(dsem, 16)
            vector.wait_ge(hsem, 16)
            d0, p0, a0 = DG_SPLIT[0]
            dg_build(0, 0, d0, vector.tensor_scalar_mul, dgsem)
            d1, p1, a1 = DG_SPLIT[1]
            dg_build(1, 0, 40, vector.tensor_scalar_mul, dgsem)

            def postproc(u):
                ct, b = u // 2, u % 2
                mc = CT_SZ[ct]
                lane, us = u % 3, u % 2
                fslot = [0, 1, 0][ct]
                vector.wait_ge(chsem, u + 1)
                vector.wait_ge(fbsems[ct], 32)
                if u >= 2:
                    vector.wait_ge(asem, u - 1)   # tmpa slot us free
                vector.scalar_tensor_tensor(
                    tmpa_sb[0:mc, us, :], acc_ps[0:mc, lane, 0:TO],
                    1.0 / TSC, fb_sb[0:mc, fslot, b, :], MU, AD).then_inc(tsem, 1)
                vector.scalar_tensor_tensor(
                    tmpb_sb[0:mc, us, :], acc_ps[0:mc, lane, 0:TO],
                    1.0 / TSC, spn_sb[0:mc, fslot, b, :], MU, MU).then_inc(gsem, 1)
                vector.tensor_reduce(rr_sb[0:mc, u, 1:2],
                                     tmpb_sb[0:mc, us, :],
                                     AXX, AD).then_inc(rsem, 1)

            dg_build(1, 40, d1, vector.tensor_scalar_mul, dgsem)
            postproc(0)
            d2, p2, a2 = DG_SPLIT[2]
            vector.wait_ge(chsem, 2)   # slot 0 free
            postproc(1)
            dg_build(2, 0, 40, vector.tensor_scalar_mul, dgsem)
            postproc(2)
            dg_build(2, 40, d2, vector.tensor_scalar_mul, dgsem)
            postproc(3)
            postproc(4)
            postproc(5)

        @block.scalar
        def _(scalar):
            scalar.wait_ge(dsem, 16)
            scalar.wait_ge(hsem, 16)
            d0, p0, a0 = DG_SPLIT[0]
            if a0:
                dg_build(0, d0 + p0, NPAIR,
                         lambda o, i, s: scalar.mul(o, i, s), dgsem)

            def spat_copy(ct):
                scalar.wait_ge(psem, ct + 1)
                if ct >= 2:
                    scalar.wait_ge(upsem, 2 * (ct - 1))  # spatj slot free
                scalar.mul(spatj_sb[0:128, ct % 2, 0:CT_SZ[ct]],
                           spat_ps[0:128, 0:CT_SZ[ct]],
                           1.0 / FSC).then_inc(scsem, 1)

            def up_copy(ct, b):
                mc = CT_SZ[ct]
                us = ct % 2
                scalar.wait_ge(upsem, 4 * ct + 2 * b + 1)
                if ct >= 2 and b == 0:
                    scalar.wait_ge(chsem, 2)  # up_sb slot0 free after units 0,1
                scalar.activation(up_sb[0:mc, us, b, 0, 0:375],
                                  up_ps[0:mc, b, 0, 0:375], CPY)
                scalar.activation(up_sb[0:mc, us, b, 1, 0:374],
                                  up_ps[0:mc, b, 0, 1:375], CPY)
                scalar.wait_ge(upsem, 4 * ct + 2 * b + 2)
                scalar.activation(up_sb[0:mc, us, b, 0, 375:750],
                                  up_ps[0:mc, b, 1, 0:375], CPY)
                scalar.activation(up_sb[0:mc, us, b, 1, 374:749],
                                  up_ps[0:mc, b, 1, 0:375],
                                  CPY).then_inc(ucsem, 1)

            def accums(u):
                ct, b = u // 2, u % 2
                mc = CT_SZ[ct]
                us = u % 2
                scalar.wait_ge(tsem, u + 1)
                scalar.activation(junk_sb[0:mc, us, :], tmpa_sb[0:mc, us, :],
                                  EXP,
                                  accum_out=rr_sb[0:mc, u, 0:1]).then_inc(asem, 1)

            spat_copy(0)
            up_copy(0, 0)
            up_copy(0, 1)
            accums(0)
            spat_copy(1)
            up_copy(1, 0)
            up_copy(1, 1)
            spat_copy(2)
            up_copy(2, 0)
            up_copy(2, 1)
            accums(1)
            accums(2)
            accums(3)
            accums(4)
            accums(5)
    return nc


_NC_CACHE = {}


def _host_prep(inputs):
    img = np.asarray(inputs["batched_image"], dtype=np.float32)
    spikes = np.asarray(inputs["batched_spikes"], dtype=np.float32)
    em = np.asarray(inputs["eye_movements"]).astype(np.int64)
    tmask = np.asarray(inputs["time_mask"], dtype=np.float32)
    sel = np.asarray(inputs["forward_sel"]).astype(np.int64)
    fw = np.asarray(inputs["forward_weights"], dtype=np.float32)
    F = np.asarray(inputs["stacked_flat_spat_filters"], dtype=np.float32)
    tcf = np.asarray(inputs["stacked_timecourse_filters"], dtype=np.float32)
    fbg = np.asarray(inputs["precomputed_feedback_gensig"], dtype=np.float32)
    histf = np.asarray(inputs["precomputed_history_frames"], dtype=np.float32)

    # jitter on host (pure gather, exact)
    jit = np.zeros((B, NJ, H, W), dtype=np.float32)
    for b in range(B):
        for f in range(NJ):
            dy, dx = int(em[b, f, 0]), int(em[b, f, 1])
            ys, xs = max(0, -dy), max(0, -dx)
            ye, xe = min(H, H - dy), min(W, W - dx)
            if ye > ys and xe > xs:
                jit[b, f, ys:ye, xs:xe] = img[b, ys + dy:ye + dy, xs + dx:xe + dx]
    jitT = jit.reshape(B, NJ, P).transpose(0, 2, 1)                 # (B,P,NJ)
    # pixel index = ch*2048 + q*128 + p
    jitp = np.zeros((B, NCH, PT, QPC, NJP), dtype=np.float32)
    jitp[..., 0:NJ] = jitT.reshape(B, NCH, QPC, PT, NJ).transpose(0, 1, 3, 2, 4)
    jitc = jitp.astype(F8)
    # jtd per batch group: (PT, NCH, QPC, BL, NJP)
    jt_h = [np.ascontiguousarray(
        np.stack([jitc[BL * bg + b] for b in range(BL)], axis=3)
        .transpose(1, 0, 2, 3, 4))
        for bg in range(GB)]

    FTf = F.T * np.float32(FSC)                                     # (P,C)
    ftc_h = {}
    for cg in range(GC):
        for ct in range(3):
            mc, off = CT_SZ[ct], CT_OFF[ct]
            X = FTf[:, cg * CL + off:cg * CL + off + mc]
            ftc_h[(cg, ct)] = np.ascontiguousarray(
                X.reshape(NCH, QPC, PT, mc).transpose(2, 0, 1, 3)
                .reshape(PT, NCH, QPC * mc)).astype(F8)

    # upsample mixing matrix M[f,t]
    Mm = np.zeros((B, NF, NB), dtype=np.float32)
    tix = np.arange(NB)
    for b in range(B):
        np.add.at(Mm[b], (sel[b, :, 0], tix), fw[b, :, 0])
        np.add.at(Mm[b], (sel[b, :, 1], tix), fw[b, :, 1])
    Mmb = Mm.astype(BF)

    mv = tmask * np.float32(MAGIC)                                  # (B,500)
    with np.errstate(divide="ignore"):
        lmv = np.log(mv).astype(np.float32)
    spn_all = -(spikes[:, :, K:] * mv[:, None, :])                  # (B,C,500)
    fb5 = fbg[:, :, :TO]
    fbl_all = fb5 + lmv[:, None, :]                                 # fb + log(mv)
    # host part of linear term: sum_{c,t} spn*fb per batch
    hconst = np.einsum('bct,bct->b', spn_all.astype(np.float64), fb5.astype(np.float64))
    histb = histf.astype(BF)                                        # (B,NH,C)
    identity = np.eye(128, dtype=np.float32).astype(BF)
    tcf_s = tcf * np.float32(TSC)

    in_maps = []
    for i in range(8):
        bg, cg = i // GC, i % GC
        bs = slice(BL * bg, BL * (bg + 1))
        cs = slice(CL * cg, CL * (cg + 1))
        fbp = np.zeros((3, 128, BL, TO), dtype=BF)
        spnp = np.zeros((3, 128, BL, TO), dtype=BF)
        for ct in range(3):
            mc, off = CT_SZ[ct], CT_OFF[ct]
            for b in range(BL):
                fbp[ct, 0:mc, b, :] = fbl_all[BL * bg + b,
                                              cg * CL + off:cg * CL + off + mc, :].astype(BF)
                spnp[ct, 0:mc, b, :] = spn_all[BL * bg + b,
                                               cg * CL + off:cg * CL + off + mc, :].astype(BF)
        auxhp = np.zeros((NH, 2200), dtype=BF)
        auxhp[:, 0:700] = histb[bs][:, :, cs].transpose(1, 0, 2).reshape(NH, 700)
        auxhp[:, 700:2200] = Mmb[bs][:, 0:NH].transpose(1, 0, 2).reshape(NH, 1500)
        auxjp = np.zeros((128, 2, NB), dtype=BF)
        auxjp[0:NJ] = Mmb[bs][:, NH:NF].transpose(1, 0, 2)
        auxjp[64:64 + NJ] = auxjp[0:NJ]
        tfp = np.zeros((128, 3, K), dtype=np.float32)
        for ct in range(3):
            mc, off = CT_SZ[ct], CT_OFF[ct]
            tfp[0:mc, ct] = tcf_s[cg * CL + off:cg * CL + off + mc]
        in_maps.append({
            "jtd": jt_h[bg],
            "ft0": ftc_h[(cg, 0)],
            "ft1": ftc_h[(cg, 1)],
            "ft2": ftc_h[(cg, 2)],
            "auxh": auxhp,
            "auxj": auxjp,
            "fbd": fbp,
            "spnd": spnp,
            "tfd": tfp,
            "ident": identity,
        })
    return in_maps, hconst


def kernel(**inputs) -> np.ndarray:
    in_maps, hconst = _host_prep(inputs)
    if "nc" not in _NC_CACHE:
        _NC_CACHE["nc"] = _build_nc()
    nc = _NC_CACHE["nc"]

    if os.environ.get("KTRACE"):
        res = run_bass_kernel_spmd(
            nc, in_maps, core_ids=list(range(8)), trace=True,
            trace_cores=[0], tmpdir=os.environ.get("KTRACE_DIR") or None)
        kernel.last_results = res
    else:
        res = run_bass_kernel_spmd(nc, in_maps, core_ids=list(range(8)))
    out = np.array(hconst, dtype=np.float64)
    for i in range(8):
        bg = i // GC
        pr = res.results[i]["part"]
        for u in range(NU):
            ct, b = u // 2, u % 2
            mc = CT_SZ[ct]
            out[BL * bg + b] += pr[0:mc, u, :].sum(dtype=np.float64)
    return out.astype(np.float32)


# revision 8
# speedup vs baseline: 1.2796x; 1.0068x over previous
import os
import numpy as np
import ml_dtypes
from contextlib import ExitStack
import concourse.bass as bass
import concourse.mybir as mybir
from concourse.ap import AP as APc
from concourse.bass_utils import run_bass_kernel_spmd

B, H, W = 8, 160, 256
C, K = 700, 250
NB = 750
NH, NJ = 30, 60
NJP = 64
NF = 90
TO = 500
P = H * W
MAGIC = 400.0 / 750.0

GB, GC = 4, 2      # batch groups x cell groups
BL = B // GB       # 2 batches per core
CL = C // GC       # 350 cells per core
PT = 128
NPT = P // PT      # 320 pixel tiles
QPC = 16           # pixel tiles per chunk
NCH = NPT // QPC   # 20 chunks
NPC = 6            # DMA pieces per (jt|ft) stream
SPANS = [(0, 4), (4, 4), (8, 4), (12, 4), (16, 3), (19, 1)]
CT_SZ = [94, 128, 128]   # smallest tile first: shortest DMA lead-in
CT_OFF = [0, 94, 222]
NPAIR = K // 2     # 125 tap pairs
NU = 6             # conv units = 3 tiles x 2 batches
FSC = 64.0
TSC = 8.0
F32 = mybir.dt.float32
BF16 = mybir.dt.bfloat16
FP8 = mybir.dt.float8e4
BF = ml_dtypes.bfloat16
F8 = ml_dtypes.float8_e4m3fn
DR = mybir.MatmulPerfMode.DoubleRow

ROW_JT = NCH * QPC * BL * NJP    # 40960 per-partition elems
ROW_FT = NCH * 2048              # ft_sb free width
# diag build split per set: (dve_pairs, pool_pairs, act_pairs)
DG_SPLIT = [(65, 30, 30), (65, 60, 0), (80, 45, 0)]
DG_SLOT = [0, 1, 0]


def _build_nc():
    CPY = mybir.ActivationFunctionType.Copy
    EXP = mybir.ActivationFunctionType.Exp
    MU = mybir.AluOpType.mult
    AD = mybir.AluOpType.add
    AXX = mybir.AxisListType.X
    nc = bass.Bass()
    jtd = nc.dram_tensor("jtd", (PT, NCH, QPC, BL, NJP), FP8, kind="ExternalInput")
    ft0 = nc.dram_tensor("ft0", (PT, NCH, QPC * CT_SZ[0]), FP8, kind="ExternalInput")
    ft1 = nc.dram_tensor("ft1", (PT, NCH, QPC * CT_SZ[1]), FP8, kind="ExternalInput")
    ft2 = nc.dram_tensor("ft2", (PT, NCH, QPC * CT_SZ[2]), FP8, kind="ExternalInput")
    auxh = nc.dram_tensor("auxh", (NH, 2200), BF16, kind="ExternalInput")
    auxj = nc.dram_tensor("auxj", (128, 2, NB), BF16, kind="ExternalInput")
    fbd = nc.dram_tensor("fbd", (3, 128, BL, TO), BF16, kind="ExternalInput")
    spnd = nc.dram_tensor("spnd", (3, 128, BL, TO), BF16, kind="ExternalInput")
    tfd = nc.dram_tensor("tfd", (128, 3, K), F32, kind="ExternalInput")
    ident = nc.dram_tensor("ident", (128, 128), BF16, kind="ExternalInput")
    part = nc.dram_tensor("part", (128, NU + 1, 2), F32, kind="ExternalOutput")
    ftd = [ft0, ft1, ft2]

    es = ExitStack()
    with es:
        jt_sb = es.enter_context(nc.sbuf_tensor("jt_sb", [PT, NCH, QPC, BL, NJP], FP8))
        ft_sb = es.enter_context(nc.sbuf_tensor("ft_sb", [PT, NCH, 2048], FP8))
        dg_sb = es.enter_context(nc.sbuf_tensor("dg_sb", [128, 2, NPAIR, 2, 128], FP8))
        ident_sb = es.enter_context(nc.sbuf_tensor("ident_sb", [128, 128], BF16))
        tf_sb = es.enter_context(nc.sbuf_tensor("tf_sb", [128, 3, K], F32))
        auxh_sb = es.enter_context(nc.sbuf_tensor("auxh_sb", [NH, 2200], BF16))
        auxj_sb = es.enter_context(nc.sbuf_tensor("auxj_sb", [128, 2, NB], BF16))
        spatj_sb = es.enter_context(nc.sbuf_tensor("spatj_sb", [128, 2, 128], BF16))
        up_sb = es.enter_context(nc.sbuf_tensor("up_sb", [128, 2, BL, 2, 752], FP8))
        fb_sb = es.enter_context(nc.sbuf_tensor("fb_sb", [128, 2, BL, TO], BF16))
        spn_sb = es.enter_context(nc.sbuf_tensor("spn_sb", [128, 2, BL, TO], BF16))
        tmpa_sb = es.enter_context(nc.sbuf_tensor("tmpa_sb", [128, 2, TO], F32))
        tmpb_sb = es.enter_context(nc.sbuf_tensor("tmpb_sb", [128, 2, TO], F32))
        junk_sb = es.enter_context(nc.sbuf_tensor("junk_sb", [128, 2, TO], F32))
        rr_sb = es.enter_context(nc.sbuf_tensor("rr_sb", [128, NU + 1, 2], F32))
        spat_ps = es.enter_context(nc.psum_tensor("spat_ps", [128, 128], F32))
        up_ps = es.enter_context(nc.psum_tensor("up_ps", [128, BL, 2, 512], F32))
        acc_ps = es.enter_context(nc.psum_tensor("acc_ps", [128, 3, 512], F32))
        jsems = [es.enter_context(nc.semaphore(f"jsem{i}")) for i in range(NPC)]
        fsems = [es.enter_context(nc.semaphore(f"fsem{i}")) for i in range(NPC)]
        psem = es.enter_context(nc.semaphore("psem"))    # spatial tile done
        scsem = es.enter_context(nc.semaphore("scsem"))  # spat copied to sbuf
        upsem = es.enter_context(nc.semaphore("upsem"))  # upsample done per (ct,b)
        ucsem = es.enter_context(nc.semaphore("ucsem"))  # up copied per (ct,b)
        dgsem = [es.enter_context(nc.semaphore(f"dgsem{i}")) for i in range(3)]
        chsem = es.enter_context(nc.semaphore("chsem"))  # conv unit done
        gsem = es.enter_context(nc.semaphore("gsem"))    # tmpb built (acc free)
        tsem = es.enter_context(nc.semaphore("tsem"))    # tmpa ready
        asem = es.enter_context(nc.semaphore("asem"))    # act accums done
        rsem = es.enter_context(nc.semaphore("rsem"))    # res ready
        osem = es.enter_context(nc.semaphore("osem"))
        osem2 = es.enter_context(nc.semaphore("osem2"))
        dsem = es.enter_context(nc.semaphore("dsem"))    # ident dma
        hsem = es.enter_context(nc.semaphore("hsem"))    # tfd dma
        asem2 = es.enter_context(nc.semaphore("asem2"))  # auxh dma
        ajs = [es.enter_context(nc.semaphore(f"ajs{i}")) for i in range(2)]
        c5sem = es.enter_context(nc.semaphore("c5sem"))  # conv5 col-halves
        fbsems = [es.enter_context(nc.semaphore(f"fbsem{i}")) for i in range(3)]
        block = es.enter_context(nc.Block())

        jt_h = jt_sb[0:PT, 0, 0, 0, 0:1]
        ft_h = ft_sb[0:PT, 0, 0:1]

        def jt_ap(ch, q):
            return APc(jt_h.tensor, ch * 2048 + q * (BL * NJP),
                       [[ROW_JT, PT], [BL * NJP, 2], [NJP, BL], [1, NJP]])

        def ft_ap(ch, q, mc):
            return APc(ft_h.tensor, ch * 2048 + q * mc,
                       [[ROW_FT, PT], [mc, 2], [1, mc]])

        @block.sync
        def _(sync):
            sync.dma_start(ident_sb[:], ident[:]).then_inc(dsem, 16)
            # lead-in: interleave jt and ft0 pieces; tfd after pair 0 (gates
            # diag builds ~14us), aux after last pair (gates upsample ~31us)
            for i, (c0, cn) in enumerate(SPANS):
                sync.dma_start(jt_sb[:, c0:c0 + cn],
                               jtd[:, c0:c0 + cn]).then_inc(jsems[i], 16)
                sync.dma_start(
                    ft_sb[:, c0:c0 + cn, 0:QPC * CT_SZ[0]],
                    ftd[0][:, c0:c0 + cn]).then_inc(fsems[i], 16)
                if i == 0:
                    sync.dma_start(tf_sb[:], tfd[:]).then_inc(hsem, 16)
            sync.dma_start(auxh_sb[:], auxh[:]).then_inc(asem2, 16)
            sync.dma_start(auxj_sb[:, 0:1], auxj[:, 0:1]).then_inc(ajs[0], 16)
            sync.dma_start(auxj_sb[:, 1:2], auxj[:, 1:2]).then_inc(ajs[1], 16)
            sync.dma_start(fb_sb[:, 0], fbd[0]).then_inc(fbsems[0], 16)
            sync.wait_ge(fbsems[0], 16)
            sync.dma_start(spn_sb[:, 0], spnd[0]).then_inc(fbsems[0], 16)
            # ft1 after spatial ct0 released ft_sb
            sync.wait_ge(psem, 1)
            for i, (c0, cn) in enumerate(SPANS):
                sync.wait_ge(fsems[i], 16)
                sync.dma_start(
                    ft_sb[:, c0:c0 + cn, 0:QPC * CT_SZ[1]],
                    ftd[1][:, c0:c0 + cn]).then_inc(fsems[i], 16)
            sync.dma_start(fb_sb[:, 1], fbd[1]).then_inc(fbsems[1], 16)
            sync.wait_ge(fbsems[1], 16)
            sync.dma_start(spn_sb[:, 1], spnd[1]).then_inc(fbsems[1], 16)
            sync.wait_ge(psem, 2)
            for i, (c0, cn) in enumerate(SPANS):
                sync.wait_ge(fsems[i], 32)
                sync.dma_start(
                    ft_sb[:, c0:c0 + cn, 0:QPC * CT_SZ[2]],
                    ftd[2][:, c0:c0 + cn]).then_inc(fsems[i], 16)
            # fb slot 0 reused for ct2: wait units 0,1 postproc done
            sync.wait_ge(gsem, 2)
            sync.dma_start(fb_sb[:, 0], fbd[2]).then_inc(fbsems[2], 16)
            sync.wait_ge(fbsems[2], 16)
            sync.dma_start(spn_sb[:, 0], spnd[2]).then_inc(fbsems[2], 16)
            # output: staging DMAs; host reassembles. First covers units
            # 0-4 (overlapped under conv5), last ships only unit 5's columns.
            sync.wait_ge(rsem, NU - 1)
            sync.wait_ge(asem, NU - 1)
            sync.dma_start(part[:, 0:NU - 1],
                           rr_sb[0:128, 0:NU - 1]).then_inc(osem, 16)
            sync.wait_ge(rsem, NU + 1)
            sync.wait_ge(asem, NU + 1)
            sync.dma_start(part[:, NU - 1:NU + 1],
                           rr_sb[0:128, NU - 1:NU + 1]).then_inc(osem2, 16)
            sync.wait_ge(osem, 16)
            sync.wait_ge(osem2, 16)

        @block.tensor
        def _(tensor):
            def spatial(ct):
                mc = CT_SZ[ct]
                if ct >= 1:
                    tensor.wait_ge(scsem, ct)   # spat_ps drained by copy
                mm = None
                for pc, (c0, cn) in enumerate(SPANS):
                    if ct == 0 and pc == 4:
                        # keep PE busy ~3us so last bursts + upsample run at
                        # full pstate (ramp needs >3us continuous execution)
                        for _ in range(30):
                            tensor.matmul(acc_ps[0:128, 0, 0:128],
                                          ident_sb[0:128, 0:128],
                                          ident_sb[0:128, 0:128],
                                          start=True, stop=True,
                                          skip_group_check=True)
                    tensor.wait_ge(fsems[pc], 16 * (ct + 1))
                    if ct == 0:
                        tensor.wait_ge(jsems[pc], 16)
                    for ch in range(c0, c0 + cn):
                        for q in range(0, QPC, 2):
                            mm = tensor.matmul(
                                spat_ps[0:128, 0:mc],
                                jt_ap(ch, q), ft_ap(ch, q, mc),
                                start=(ch == 0 and q == 0),
                                stop=(ch == NCH - 1 and q == QPC - 2),
                                skip_group_check=True, perf_mode=DR)
                mm.then_inc(psem, 1)

            def upsample(ct):
                mc, off = CT_SZ[ct], CT_OFF[ct]
                if ct == 0:
                    tensor.wait_ge(asem2, 16)
                tensor.wait_ge(scsem, ct + 1)
                if ct >= 1:
                    tensor.wait_ge(ucsem, 2 * ct)   # up_ps drained by copies
                for b in range(BL):
                    if ct == 0:
                        tensor.wait_ge(ajs[b], 16)
                    for h in range(2):
                        tensor.matmul(up_ps[0:mc, b, h, 0:375],
                                      auxh_sb[0:NH, 350 * b + off:350 * b + off + mc],
                                      auxh_sb[0:NH, 700 + 750 * b + 375 * h:
                                              700 + 750 * b + 375 * (h + 1)],
                                      start=True, stop=False,
                                      skip_group_check=True)
                        tensor.matmul(
                            up_ps[0:mc, b, h, 0:375],
                            spatj_sb[64 * b:64 * b + 60, ct % 2, 0:mc],
                            auxj_sb[64 * b:64 * b + 60, b, 375 * h:375 * (h + 1)],
                            start=False, stop=True,
                            skip_group_check=True).then_inc(upsem, 1)

            def conv(u, p0=0, p1=NPAIR):
                ct, b = u // 2, u % 2
                mc = CT_SZ[ct]
                lane = u % 3
                if p0 == 0:
                    tensor.wait_ge(dgsem[ct], 2 * NPAIR)
                    tensor.wait_ge(ucsem, 2 * ct + b + 1)
                    if u >= 3:
                        tensor.wait_ge(gsem, u - 2)
                mm = None
                for p in range(p0, p1):
                    mm = tensor.matmul(
                        acc_ps[0:mc, lane, 0:TO],
                        dg_sb[0:mc, DG_SLOT[ct], p, 0:2, 0:mc],
                        up_sb[0:mc, ct % 2, b, 0:2, 2 * p:2 * p + TO],
                        start=(p == 0), stop=(p == NPAIR - 1),
                        skip_group_check=True, perf_mode=DR)
                if p1 == NPAIR:
                    mm.then_inc(chsem, 1)

            spatial(0)
            upsample(0)
            conv(0)
            spatial(1)
            conv(1, 0, 62)
            upsample(1)
            conv(1, 62, NPAIR)
            conv(2)
            spatial(2)
            conv(3, 0, 62)
            upsample(2)
            conv(3, 62, NPAIR)
            conv(4)
            mc5 = CT_SZ[2]
            tensor.wait_ge(dgsem[2], 2 * NPAIR)
            tensor.wait_ge(ucsem, 6)
            tensor.wait_ge(gsem, 3)
            for half in range(2):
                c0 = 250 * half
                lane5 = 2 if half == 0 else 0
                if half == 1:
                    tensor.wait_ge(gsem, 4)   # lane 0 freed by pp3
                mm = None
                for p in range(NPAIR):
                    mm = tensor.matmul(
                        acc_ps[0:mc5, lane5, 0:250],
                        dg_sb[0:mc5, 0, p, 0:2, 0:mc5],
                        up_sb[0:mc5, 0, 1, 0:2, 2 * p + c0:2 * p + c0 + 250],
                        start=(p == 0), stop=(p == NPAIR - 1),
                        skip_group_check=True, perf_mode=DR)
                mm.then_inc(c5sem, 1)

        def dg_build(ct, p0, p1, ts_fn, sem_target):
            mc = CT_SZ[ct]
            for p in range(p0, p1):
                for j in range(2):
                    col = 2 * p + j
                    ts_fn(dg_sb[0:mc, DG_SLOT[ct], p, j, 0:mc],
                          ident_sb[0:mc, 0:mc],
                          tf_sb[0:mc, ct, col:col + 1]).then_inc(sem_target[ct], 1)

        @block.gpsimd
        def _(gp):
            gp.wait_ge(dsem, 16)
            gp.wait_ge(hsem, 16)
            d0, p0, a0 = DG_SPLIT[0]
            dg_build(0, d0, d0 + p0, gp.tensor_scalar_mul, dgsem)
            d1, p1, a1 = DG_SPLIT[1]
            dg_build(1, d1, d1 + p1, gp.tensor_scalar_mul, dgsem)
            gp.wait_ge(chsem, 2)   # slot 0 free after conv units 0,1
            d2, p2, a2 = DG_SPLIT[2]
            dg_build(2, d2, d2 + p2, gp.tensor_scalar_mul, dgsem)


        @block.vector
        def _(vector):
            vector.wait_ge(dsem, 16)
            vector.wait_ge(hsem, 16)
            d0, p0, a0 = DG_SPLIT[0]
            dg_build(0, 0, d0, vector.tensor_scalar_mul, dgsem)
            d1, p1, a1 = DG_SPLIT[1]
            dg_build(1, 0, 40, vector.tensor_scalar_mul, dgsem)

            def postproc(u):
                ct, b = u // 2, u % 2
                mc = CT_SZ[ct]
                lane, us = u % 3, u % 2
                fslot = [0, 1, 0][ct]
                vector.wait_ge(chsem, u + 1)
                vector.wait_ge(fbsems[ct], 32)
                if u >= 2:
                    vector.wait_ge(asem, u - 1)   # tmpa slot us free
                vector.scalar_tensor_tensor(
                    tmpa_sb[0:mc, us, :], acc_ps[0:mc, lane, 0:TO],
                    1.0 / TSC, fb_sb[0:mc, fslot, b, :], MU, AD).then_inc(tsem, 1)
                vector.scalar_tensor_tensor(
                    tmpb_sb[0:mc, us, :], acc_ps[0:mc, lane, 0:TO],
                    1.0 / TSC, spn_sb[0:mc, fslot, b, :], MU, MU).then_inc(gsem, 1)
                vector.tensor_reduce(rr_sb[0:mc, u, 1:2],
                                     tmpb_sb[0:mc, us, :],
                                     AXX, AD).then_inc(rsem, 1)

            dg_build(1, 40, d1, vector.tensor_scalar_mul, dgsem)
            postproc(0)
            d2, p2, a2 = DG_SPLIT[2]
            vector.wait_ge(chsem, 2)   # slot 0 free
            postproc(1)
            dg_build(2, 0, 40, vector.tensor_scalar_mul, dgsem)
            postproc(2)
            dg_build(2, 40, d2, vector.tensor_scalar_mul, dgsem)
            postproc(3)
            postproc(4)
            for half in range(2):
                mc5 = CT_SZ[2]
                c0 = 250 * half
                lane5 = 2 if half == 0 else 0
                vector.wait_ge(c5sem, half + 1)
                if half == 0:
                    vector.wait_ge(fbsems[2], 32)
                    vector.wait_ge(asem, 4)
                vector.scalar_tensor_tensor(
                    tmpa_sb[0:mc5, 1, c0:c0 + 250],
                    acc_ps[0:mc5, lane5, 0:250],
                    1.0 / TSC, fb_sb[0:mc5, 0, 1, c0:c0 + 250],
                    MU, AD).then_inc(tsem, 1)
                vector.scalar_tensor_tensor(
                    tmpb_sb[0:mc5, 1, c0:c0 + 250],
                    acc_ps[0:mc5, lane5, 0:250],
                    1.0 / TSC, spn_sb[0:mc5, 0, 1, c0:c0 + 250], MU, MU)
                vector.tensor_reduce(rr_sb[0:mc5, 5 + half, 1:2],
                                     tmpb_sb[0:mc5, 1, c0:c0 + 250],
                                     AXX, AD).then_inc(rsem, 1)

        @block.scalar
        def _(scalar):
            scalar.wait_ge(dsem, 16)
            scalar.wait_ge(hsem, 16)
            d0, p0, a0 = DG_SPLIT[0]
            if a0:
                dg_build(0, d0 + p0, NPAIR,
                         lambda o, i, s: scalar.mul(o, i, s), dgsem)

            def spat_copy(ct):
                scalar.wait_ge(psem, ct + 1)
                if ct >= 2:
                    scalar.wait_ge(upsem, 2 * (ct - 1))  # spatj slot free
                scalar.mul(spatj_sb[0:128, ct % 2, 0:CT_SZ[ct]],
                           spat_ps[0:128, 0:CT_SZ[ct]],
                           1.0 / FSC).then_inc(scsem, 1)

            def up_copy(ct, b):
                mc = CT_SZ[ct]
                us = ct % 2
                scalar.wait_ge(upsem, 4 * ct + 2 * b + 1)
                if ct >= 2 and b == 0:
                    scalar.wait_ge(chsem, 2)  # up_sb slot0 free after units 0,1
                scalar.activation(up_sb[0:mc, us, b, 0, 0:375],
                                  up_ps[0:mc, b, 0, 0:375], CPY)
                scalar.activation(up_sb[0:mc, us, b, 1, 0:374],
                                  up_ps[0:mc, b, 0, 1:375], CPY)
                scalar.wait_ge(upsem, 4 * ct + 2 * b + 2)
                scalar.activation(up_sb[0:mc, us, b, 0, 375:750],
                                  up_ps[0:mc, b, 1, 0:375], CPY)
                scalar.activation(up_sb[0:mc, us, b, 1, 374:749],
                                  up_ps[0:mc, b, 1, 0:375],
                                  CPY).then_inc(ucsem, 1)

            def accums(u):
                ct, b = u // 2, u % 2
                mc = CT_SZ[ct]
                us = u % 2
                scalar.wait_ge(tsem, u + 1)
                scalar.activation(junk_sb[0:mc, us, :], tmpa_sb[0:mc, us, :],
                                  EXP,
                                  accum_out=rr_sb[0:mc, u, 0:1]).then_inc(asem, 1)

            spat_copy(0)
            up_copy(0, 0)
            up_copy(0, 1)
            accums(0)
            spat_copy(1)
            up_copy(1, 0)
            up_copy(1, 1)
            spat_copy(2)
            up_copy(2, 0)
            up_copy(2, 1)
            accums(1)
            accums(2)
            accums(3)
            accums(4)
            for half in range(2):
                mc5 = CT_SZ[2]
                c0 = 250 * half
                scalar.wait_ge(tsem, 6 + half)
                scalar.activation(junk_sb[0:mc5, 1, c0:c0 + 250],
                                  tmpa_sb[0:mc5, 1, c0:c0 + 250], EXP,
                                  accum_out=rr_sb[0:mc5, 5 + half,
                                                  0:1]).then_inc(asem, 1)
    return nc


_NC_CACHE = {}


def _host_prep(inputs):
    img = np.asarray(inputs["batched_image"], dtype=np.float32)
    spikes = np.asarray(inputs["batched_spikes"], dtype=np.float32)
    em = np.asarray(inputs["eye_movements"]).astype(np.int64)
    tmask = np.asarray(inputs["time_mask"], dtype=np.float32)
    sel = np.asarray(inputs["forward_sel"]).astype(np.int64)
    fw = np.asarray(inputs["forward_weights"], dtype=np.float32)
    F = np.asarray(inputs["stacked_flat_spat_filters"], dtype=np.float32)
    tcf = np.asarray(inputs["stacked_timecourse_filters"], dtype=np.float32)
    fbg = np.asarray(inputs["precomputed_feedback_gensig"], dtype=np.float32)
    histf = np.asarray(inputs["precomputed_history_frames"], dtype=np.float32)

    # jitter on host (pure gather, exact)
    jit = np.zeros((B, NJ, H, W), dtype=np.float32)
    for b in range(B):
        for f in range(NJ):
            dy, dx = int(em[b, f, 0]), int(em[b, f, 1])
            ys, xs = max(0, -dy), max(0, -dx)
            ye, xe = min(H, H - dy), min(W, W - dx)
            if ye > ys and xe > xs:
                jit[b, f, ys:ye, xs:xe] = img[b, ys + dy:ye + dy, xs + dx:xe + dx]
    jitT = jit.reshape(B, NJ, P).transpose(0, 2, 1)                 # (B,P,NJ)
    # pixel index = ch*2048 + q*128 + p
    jitp = np.zeros((B, NCH, PT, QPC, NJP), dtype=np.float32)
    jitp[..., 0:NJ] = jitT.reshape(B, NCH, QPC, PT, NJ).transpose(0, 1, 3, 2, 4)
    jitc = jitp.astype(F8)
    # jtd per batch group: (PT, NCH, QPC, BL, NJP)
    jt_h = [np.ascontiguousarray(
        np.stack([jitc[BL * bg + b] for b in range(BL)], axis=3)
        .transpose(1, 0, 2, 3, 4))
        for bg in range(GB)]

    FTf = F.T * np.float32(FSC)                                     # (P,C)
    ftc_h = {}
    for cg in range(GC):
        for ct in range(3):
            mc, off = CT_SZ[ct], CT_OFF[ct]
            X = FTf[:, cg * CL + off:cg * CL + off + mc]
            ftc_h[(cg, ct)] = np.ascontiguousarray(
                X.reshape(NCH, QPC, PT, mc).transpose(2, 0, 1, 3)
                .reshape(PT, NCH, QPC * mc)).astype(F8)

    # upsample mixing matrix M[f,t]
    Mm = np.zeros((B, NF, NB), dtype=np.float32)
    tix = np.arange(NB)
    for b in range(B):
        np.add.at(Mm[b], (sel[b, :, 0], tix), fw[b, :, 0])
        np.add.at(Mm[b], (sel[b, :, 1], tix), fw[b, :, 1])
    Mmb = Mm.astype(BF)

    mv = tmask * np.float32(MAGIC)                                  # (B,500)
    with np.errstate(divide="ignore"):
        lmv = np.log(mv).astype(np.float32)
    spn_all = -(spikes[:, :, K:] * mv[:, None, :])                  # (B,C,500)
    fb5 = fbg[:, :, :TO]
    fbl_all = fb5 + lmv[:, None, :]                                 # fb + log(mv)
    # host part of linear term: sum_{c,t} spn*fb per batch
    hconst = np.einsum('bct,bct->b', spn_all.astype(np.float64), fb5.astype(np.float64))
    histb = histf.astype(BF)                                        # (B,NH,C)
    identity = np.eye(128, dtype=np.float32).astype(BF)
    tcf_s = tcf * np.float32(TSC)

    in_maps = []
    for i in range(8):
        bg, cg = i // GC, i % GC
        bs = slice(BL * bg, BL * (bg + 1))
        cs = slice(CL * cg, CL * (cg + 1))
        fbp = np.zeros((3, 128, BL, TO), dtype=BF)
        spnp = np.zeros((3, 128, BL, TO), dtype=BF)
        for ct in range(3):
            mc, off = CT_SZ[ct], CT_OFF[ct]
            for b in range(BL):
                fbp[ct, 0:mc, b, :] = fbl_all[BL * bg + b,
                                              cg * CL + off:cg * CL + off + mc, :].astype(BF)
                spnp[ct, 0:mc, b, :] = spn_all[BL * bg + b,
                                               cg * CL + off:cg * CL + off + mc, :].astype(BF)
        auxhp = np.zeros((NH, 2200), dtype=BF)
        auxhp[:, 0:700] = histb[bs][:, :, cs].transpose(1, 0, 2).reshape(NH, 700)
        auxhp[:, 700:2200] = Mmb[bs][:, 0:NH].transpose(1, 0, 2).reshape(NH, 1500)
        auxjp = np.zeros((128, 2, NB), dtype=BF)
        auxjp[0:NJ] = Mmb[bs][:, NH:NF].transpose(1, 0, 2)
        auxjp[64:64 + NJ] = auxjp[0:NJ]
        tfp = np.zeros((128, 3, K), dtype=np.float32)
        for ct in range(3):
            mc, off = CT_SZ[ct], CT_OFF[ct]
            tfp[0:mc, ct] = tcf_s[cg * CL + off:cg * CL + off + mc]
        in_maps.append({
            "jtd": jt_h[bg],
            "ft0": ftc_h[(cg, 0)],
            "ft1": ftc_h[(cg, 1)],
            "ft2": ftc_h[(cg, 2)],
            "auxh": auxhp,
            "auxj": auxjp,
            "fbd": fbp,
            "spnd": spnp,
            "tfd": tfp,
            "ident": identity,
        })
    return in_maps, hconst


def kernel(**inputs) -> np.ndarray:
    in_maps, hconst = _host_prep(inputs)
    if "nc" not in _NC_CACHE:
        _NC_CACHE["nc"] = _build_nc()
    nc = _NC_CACHE["nc"]

    if os.environ.get("KTRACE"):
        res = run_bass_kernel_spmd(
            nc, in_maps, core_ids=list(range(8)), trace=True,
            trace_cores=[0], tmpdir=os.environ.get("KTRACE_DIR") or None)
        kernel.last_results = res
    else:
        res = run_bass_kernel_spmd(nc, in_maps, core_ids=list(range(8)))
    out = np.array(hconst, dtype=np.float64)
    for i in range(8):
        bg = i // GC
        pr = res.results[i]["part"]
        for u in range(NU):
            ct, b = u // 2, u % 2
            mc = CT_SZ[ct]
            u1 = u + 2 if u == NU - 1 else u + 1
            out[BL * bg + b] += pr[0:mc, u:u1, :].sum(dtype=np.float64)
    return out.astype(np.float32)


# revision 9
# speedup vs baseline: 1.2827x; 1.0024x over previous
import os
import numpy as np
import ml_dtypes
from contextlib import ExitStack
import concourse.bass as bass
import concourse.mybir as mybir
from concourse.ap import AP as APc
from concourse.bass_utils import run_bass_kernel_spmd

B, H, W = 8, 160, 256
C, K = 700, 250
NB = 750
NH, NJ = 30, 60
NJP = 64
NF = 90
TO = 500
P = H * W
MAGIC = 400.0 / 750.0

GB, GC = 4, 2      # batch groups x cell groups
BL = B // GB       # 2 batches per core
CL = C // GC       # 350 cells per core
PT = 128
NPT = P // PT      # 320 pixel tiles
QPC = 16           # pixel tiles per chunk
NCH = NPT // QPC   # 20 chunks
NPC = 6            # DMA pieces per (jt|ft) stream
SPANS = [(0, 4), (4, 4), (8, 4), (12, 4), (16, 3), (19, 1)]
CT_SZ = [94, 128, 128]   # smallest tile first: shortest DMA lead-in
CT_OFF = [0, 94, 222]
NPAIR = K // 2     # 125 tap pairs
NU = 6             # conv units = 3 tiles x 2 batches
FSC = 64.0
TSC = 8.0
F32 = mybir.dt.float32
BF16 = mybir.dt.bfloat16
FP8 = mybir.dt.float8e4
BF = ml_dtypes.bfloat16
F8 = ml_dtypes.float8_e4m3fn
DR = mybir.MatmulPerfMode.DoubleRow

ROW_JT = NCH * QPC * BL * NJP    # 40960 per-partition elems
ROW_FT = NCH * 2048              # ft_sb free width
# diag build split per set: (dve_pairs, pool_pairs, act_pairs)
DG_SPLIT = [(65, 30, 30), (65, 60, 0), (80, 45, 0)]
DG_SLOT = [0, 1, 0]


def _build_nc():
    CPY = mybir.ActivationFunctionType.Copy
    EXP = mybir.ActivationFunctionType.Exp
    MU = mybir.AluOpType.mult
    AD = mybir.AluOpType.add
    AXX = mybir.AxisListType.X
    nc = bass.Bass()
    jtd = nc.dram_tensor("jtd", (PT, NCH, QPC, BL, NJP), FP8, kind="ExternalInput")
    ft0 = nc.dram_tensor("ft0", (PT, NCH, QPC * CT_SZ[0]), FP8, kind="ExternalInput")
    ft1 = nc.dram_tensor("ft1", (PT, NCH, QPC * CT_SZ[1]), FP8, kind="ExternalInput")
    ft2 = nc.dram_tensor("ft2", (PT, NCH, QPC * CT_SZ[2]), FP8, kind="ExternalInput")
    auxh = nc.dram_tensor("auxh", (NH, 2200), BF16, kind="ExternalInput")
    auxj = nc.dram_tensor("auxj", (128, 2, NB), BF16, kind="ExternalInput")
    fbd = nc.dram_tensor("fbd", (3, 128, BL, TO), BF16, kind="ExternalInput")
    spnd = nc.dram_tensor("spnd", (3, 128, BL, TO), BF16, kind="ExternalInput")
    tfd = nc.dram_tensor("tfd", (128, 3, K), F32, kind="ExternalInput")
    ident = nc.dram_tensor("ident", (128, 128), BF16, kind="ExternalInput")
    part = nc.dram_tensor("part", (128, NU + 1, 2), F32, kind="ExternalOutput")
    ftd = [ft0, ft1, ft2]

    es = ExitStack()
    with es:
        jt_sb = es.enter_context(nc.sbuf_tensor("jt_sb", [PT, NCH, QPC, BL, NJP], FP8))
        ft_sb = es.enter_context(nc.sbuf_tensor("ft_sb", [PT, NCH, 2048], FP8))
        dg_sb = es.enter_context(nc.sbuf_tensor("dg_sb", [128, 2, NPAIR, 2, 128], FP8))
        ident_sb = es.enter_context(nc.sbuf_tensor("ident_sb", [128, 128], BF16))
        tf_sb = es.enter_context(nc.sbuf_tensor("tf_sb", [128, 3, K], F32))
        auxh_sb = es.enter_context(nc.sbuf_tensor("auxh_sb", [NH, 2200], BF16))
        auxj_sb = es.enter_context(nc.sbuf_tensor("auxj_sb", [128, 2, NB], BF16))
        spatj_sb = es.enter_context(nc.sbuf_tensor("spatj_sb", [128, 2, 128], BF16))
        up_sb = es.enter_context(nc.sbuf_tensor("up_sb", [128, 2, BL, 2, 752], FP8))
        fb_sb = es.enter_context(nc.sbuf_tensor("fb_sb", [128, 2, BL, TO], BF16))
        spn_sb = es.enter_context(nc.sbuf_tensor("spn_sb", [128, 2, BL, TO], BF16))
        tmpa_sb = es.enter_context(nc.sbuf_tensor("tmpa_sb", [128, 2, TO], F32))
        tmpb_sb = es.enter_context(nc.sbuf_tensor("tmpb_sb", [128, 2, TO], F32))
        junk_sb = es.enter_context(nc.sbuf_tensor("junk_sb", [128, 2, TO], F32))
        rr_sb = es.enter_context(nc.sbuf_tensor("rr_sb", [128, NU + 1, 2], F32))
        spat_ps = es.enter_context(nc.psum_tensor("spat_ps", [128, 128], F32))
        up_ps = es.enter_context(nc.psum_tensor("up_ps", [128, BL, 2, 512], F32))
        acc_ps = es.enter_context(nc.psum_tensor("acc_ps", [128, 3, 512], F32))
        jsems = [es.enter_context(nc.semaphore(f"jsem{i}")) for i in range(NPC)]
        fsems = [es.enter_context(nc.semaphore(f"fsem{i}")) for i in range(NPC)]
        psem = es.enter_context(nc.semaphore("psem"))    # spatial tile done
        scsem = es.enter_context(nc.semaphore("scsem"))  # spat copied to sbuf
        upsem = es.enter_context(nc.semaphore("upsem"))  # upsample done per (ct,b)
        ucsem = es.enter_context(nc.semaphore("ucsem"))  # up copied per (ct,b)
        dgsem = [es.enter_context(nc.semaphore(f"dgsem{i}")) for i in range(3)]
        chsem = es.enter_context(nc.semaphore("chsem"))  # conv unit done
        gsem = es.enter_context(nc.semaphore("gsem"))    # tmpb built (acc free)
        tsem = es.enter_context(nc.semaphore("tsem"))    # tmpa ready
        asem = es.enter_context(nc.semaphore("asem"))    # act accums done
        rsem = es.enter_context(nc.semaphore("rsem"))    # res ready
        osem = es.enter_context(nc.semaphore("osem"))
        osem2 = es.enter_context(nc.semaphore("osem2"))
        dsem = es.enter_context(nc.semaphore("dsem"))    # ident dma
        hsem = es.enter_context(nc.semaphore("hsem"))    # tfd dma
        asem2 = es.enter_context(nc.semaphore("asem2"))  # auxh dma
        ajs = [es.enter_context(nc.semaphore(f"ajs{i}")) for i in range(2)]
        c5sem = es.enter_context(nc.semaphore("c5sem"))  # conv5 col-halves
        fbsems = [es.enter_context(nc.semaphore(f"fbsem{i}")) for i in range(3)]
        block = es.enter_context(nc.Block())

        jt_h = jt_sb[0:PT, 0, 0, 0, 0:1]
        ft_h = ft_sb[0:PT, 0, 0:1]

        def jt_ap(ch, q):
            return APc(jt_h.tensor, ch * 2048 + q * (BL * NJP),
                       [[ROW_JT, PT], [BL * NJP, 2], [NJP, BL], [1, NJP]])

        def ft_ap(ch, q, mc):
            return APc(ft_h.tensor, ch * 2048 + q * mc,
                       [[ROW_FT, PT], [mc, 2], [1, mc]])

        @block.sync
        def _(sync):
            sync.dma_start(ident_sb[:], ident[:]).then_inc(dsem, 16)
            # lead-in: interleave jt and ft0 pieces; tfd after pair 0 (gates
            # diag builds ~14us), aux after last pair (gates upsample ~31us)
            for i, (c0, cn) in enumerate(SPANS):
                sync.dma_start(jt_sb[:, c0:c0 + cn],
                               jtd[:, c0:c0 + cn]).then_inc(jsems[i], 16)
                sync.dma_start(
                    ft_sb[:, c0:c0 + cn, 0:QPC * CT_SZ[0]],
                    ftd[0][:, c0:c0 + cn]).then_inc(fsems[i], 16)
                if i == 0:
                    sync.dma_start(tf_sb[:], tfd[:]).then_inc(hsem, 16)
            sync.dma_start(auxh_sb[:], auxh[:]).then_inc(asem2, 16)
            sync.dma_start(auxj_sb[:, 0:1], auxj[:, 0:1]).then_inc(ajs[0], 16)
            sync.dma_start(auxj_sb[:, 1:2], auxj[:, 1:2]).then_inc(ajs[1], 16)
            sync.dma_start(fb_sb[:, 0], fbd[0]).then_inc(fbsems[0], 16)
            sync.wait_ge(fbsems[0], 16)
            sync.dma_start(spn_sb[:, 0], spnd[0]).then_inc(fbsems[0], 16)
            # ft1 after spatial ct0 released ft_sb
            sync.wait_ge(psem, 1)
            for i, (c0, cn) in enumerate(SPANS):
                sync.wait_ge(fsems[i], 16)
                sync.dma_start(
                    ft_sb[:, c0:c0 + cn, 0:QPC * CT_SZ[1]],
                    ftd[1][:, c0:c0 + cn]).then_inc(fsems[i], 16)
            sync.dma_start(fb_sb[:, 1], fbd[1]).then_inc(fbsems[1], 16)
            sync.wait_ge(fbsems[1], 16)
            sync.dma_start(spn_sb[:, 1], spnd[1]).then_inc(fbsems[1], 16)
            sync.wait_ge(psem, 2)
            for i, (c0, cn) in enumerate(SPANS):
                sync.wait_ge(fsems[i], 32)
                sync.dma_start(
                    ft_sb[:, c0:c0 + cn, 0:QPC * CT_SZ[2]],
                    ftd[2][:, c0:c0 + cn]).then_inc(fsems[i], 16)
            # fb slot 0 reused for ct2: wait units 0,1 postproc done
            sync.wait_ge(gsem, 2)
            sync.dma_start(fb_sb[:, 0], fbd[2]).then_inc(fbsems[2], 16)
            sync.wait_ge(fbsems[2], 16)
            sync.dma_start(spn_sb[:, 0], spnd[2]).then_inc(fbsems[2], 16)
            # output: staging DMAs; host reassembles. First covers units
            # 0-4 (overlapped under conv5), last ships only unit 5's columns.
            sync.wait_ge(rsem, NU - 1)
            sync.wait_ge(asem, NU - 1)
            sync.dma_start(part[:, 0:NU - 1],
                           rr_sb[0:128, 0:NU - 1]).then_inc(osem, 16)
            sync.wait_ge(rsem, NU + 1)
            sync.wait_ge(asem, NU + 1)
            sync.dma_start(part[:, NU - 1:NU + 1],
                           rr_sb[0:128, NU - 1:NU + 1]).then_inc(osem2, 16)

        @block.tensor
        def _(tensor):
            def spatial(ct):
                mc = CT_SZ[ct]
                if ct >= 1:
                    tensor.wait_ge(scsem, ct)   # spat_ps drained by copy
                mm = None
                for pc, (c0, cn) in enumerate(SPANS):
                    if ct == 0 and pc == 4:
                        # keep PE busy ~3us so last bursts + upsample run at
                        # full pstate (ramp needs >3us continuous execution)
                        for _ in range(30):
                            tensor.matmul(acc_ps[0:128, 0, 0:128],
                                          ident_sb[0:128, 0:128],
                                          ident_sb[0:128, 0:128],
                                          start=True, stop=True,
                                          skip_group_check=True)
                    tensor.wait_ge(fsems[pc], 16 * (ct + 1))
                    if ct == 0:
                        tensor.wait_ge(jsems[pc], 16)
                    for ch in range(c0, c0 + cn):
                        for q in range(0, QPC, 2):
                            mm = tensor.matmul(
                                spat_ps[0:128, 0:mc],
                                jt_ap(ch, q), ft_ap(ch, q, mc),
                                start=(ch == 0 and q == 0),
                                stop=(ch == NCH - 1 and q == QPC - 2),
                                skip_group_check=True, perf_mode=DR)
                mm.then_inc(psem, 1)

            def upsample(ct):
                mc, off = CT_SZ[ct], CT_OFF[ct]
                if ct == 0:
                    tensor.wait_ge(asem2, 16)
                tensor.wait_ge(scsem, ct + 1)
                if ct >= 1:
                    tensor.wait_ge(ucsem, 2 * ct)   # up_ps drained by copies
                for b in range(BL):
                    if ct == 0:
                        tensor.wait_ge(ajs[b], 16)
                    for h in range(2):
                        tensor.matmul(up_ps[0:mc, b, h, 0:375],
                                      auxh_sb[0:NH, 350 * b + off:350 * b + off + mc],
                                      auxh_sb[0:NH, 700 + 750 * b + 375 * h:
                                              700 + 750 * b + 375 * (h + 1)],
                                      start=True, stop=False,
                                      skip_group_check=True)
                        tensor.matmul(
                            up_ps[0:mc, b, h, 0:375],
                            spatj_sb[64 * b:64 * b + 60, ct % 2, 0:mc],
                            auxj_sb[64 * b:64 * b + 60, b, 375 * h:375 * (h + 1)],
                            start=False, stop=True,
                            skip_group_check=True).then_inc(upsem, 1)

            def conv(u, p0=0, p1=NPAIR):
                ct, b = u // 2, u % 2
                mc = CT_SZ[ct]
                lane = u % 3
                if p0 == 0:
                    tensor.wait_ge(dgsem[ct], 2 * NPAIR)
                    tensor.wait_ge(ucsem, 2 * ct + b + 1)
                    if u >= 3:
                        tensor.wait_ge(gsem, u - 2)
                mm = None
                for p in range(p0, p1):
                    mm = tensor.matmul(
                        acc_ps[0:mc, lane, 0:TO],
                        dg_sb[0:mc, DG_SLOT[ct], p, 0:2, 0:mc],
                        up_sb[0:mc, ct % 2, b, 0:2, 2 * p:2 * p + TO],
                        start=(p == 0), stop=(p == NPAIR - 1),
                        skip_group_check=True, perf_mode=DR)
                if p1 == NPAIR:
                    mm.then_inc(chsem, 1)

            spatial(0)
            upsample(0)
            conv(0)
            spatial(1)
            conv(1, 0, 62)
            upsample(1)
            conv(1, 62, NPAIR)
            conv(2)
            spatial(2)
            conv(3, 0, 62)
            upsample(2)
            conv(3, 62, NPAIR)
            conv(4)
            mc5 = CT_SZ[2]
            tensor.wait_ge(dgsem[2], 2 * NPAIR)
            tensor.wait_ge(ucsem, 6)
            tensor.wait_ge(gsem, 3)
            for half in range(2):
                c0 = 250 * half
                lane5 = 2 if half == 0 else 0
                if half == 1:
                    tensor.wait_ge(gsem, 4)   # lane 0 freed by pp3
                mm = None
                for p in range(NPAIR):
                    mm = tensor.matmul(
                        acc_ps[0:mc5, lane5, 0:250],
                        dg_sb[0:mc5, 0, p, 0:2, 0:mc5],
                        up_sb[0:mc5, 0, 1, 0:2, 2 * p + c0:2 * p + c0 + 250],
                        start=(p == 0), stop=(p == NPAIR - 1),
                        skip_group_check=True, perf_mode=DR)
                mm.then_inc(c5sem, 1)

        def dg_build(ct, p0, p1, ts_fn, sem_target):
            mc = CT_SZ[ct]
            for p in range(p0, p1):
                for j in range(2):
                    col = 2 * p + j
                    ts_fn(dg_sb[0:mc, DG_SLOT[ct], p, j, 0:mc],
                          ident_sb[0:mc, 0:mc],
                          tf_sb[0:mc, ct, col:col + 1]).then_inc(sem_target[ct], 1)

        @block.gpsimd
        def _(gp):
            gp.wait_ge(dsem, 16)
            gp.wait_ge(hsem, 16)
            d0, p0, a0 = DG_SPLIT[0]
            dg_build(0, d0, d0 + p0, gp.tensor_scalar_mul, dgsem)
            d1, p1, a1 = DG_SPLIT[1]
            dg_build(1, d1, d1 + p1, gp.tensor_scalar_mul, dgsem)
            gp.wait_ge(chsem, 2)   # slot 0 free after conv units 0,1
            d2, p2, a2 = DG_SPLIT[2]
            dg_build(2, d2, d2 + p2, gp.tensor_scalar_mul, dgsem)


        @block.vector
        def _(vector):
            vector.wait_ge(dsem, 16)
            vector.wait_ge(hsem, 16)
            d0, p0, a0 = DG_SPLIT[0]
            dg_build(0, 0, d0, vector.tensor_scalar_mul, dgsem)
            d1, p1, a1 = DG_SPLIT[1]
            dg_build(1, 0, 40, vector.tensor_scalar_mul, dgsem)

            def postproc(u):
                ct, b = u // 2, u % 2
                mc = CT_SZ[ct]
                lane, us = u % 3, u % 2
                fslot = [0, 1, 0][ct]
                vector.wait_ge(chsem, u + 1)
                vector.wait_ge(fbsems[ct], 32)
                if u >= 2:
                    vector.wait_ge(asem, u - 1)   # tmpa slot us free
                vector.scalar_tensor_tensor(
                    tmpa_sb[0:mc, us, :], acc_ps[0:mc, lane, 0:TO],
                    1.0 / TSC, fb_sb[0:mc, fslot, b, :], MU, AD).then_inc(tsem, 1)
                vector.scalar_tensor_tensor(
                    tmpb_sb[0:mc, us, :], acc_ps[0:mc, lane, 0:TO],
                    1.0 / TSC, spn_sb[0:mc, fslot, b, :], MU, MU).then_inc(gsem, 1)
                vector.tensor_reduce(rr_sb[0:mc, u, 1:2],
                                     tmpb_sb[0:mc, us, :],
                                     AXX, AD).then_inc(rsem, 1)

            dg_build(1, 40, d1, vector.tensor_scalar_mul, dgsem)
            postproc(0)
            d2, p2, a2 = DG_SPLIT[2]
            vector.wait_ge(chsem, 2)   # slot 0 free
            postproc(1)
            dg_build(2, 0, 40, vector.tensor_scalar_mul, dgsem)
            postproc(2)
            dg_build(2, 40, d2, vector.tensor_scalar_mul, dgsem)
            postproc(3)
            postproc(4)
            for half in range(2):
                mc5 = CT_SZ[2]
                c0 = 250 * half
                lane5 = 2 if half == 0 else 0
                vector.wait_ge(c5sem, half + 1)
                if half == 0:
                    vector.wait_ge(fbsems[2], 32)
                    vector.wait_ge(asem, 4)
                vector.scalar_tensor_tensor(
                    tmpa_sb[0:mc5, 1, c0:c0 + 250],
                    acc_ps[0:mc5, lane5, 0:250],
                    1.0 / TSC, fb_sb[0:mc5, 0, 1, c0:c0 + 250],
                    MU, AD).then_inc(tsem, 1)
                vector.scalar_tensor_tensor(
                    tmpb_sb[0:mc5, 1, c0:c0 + 250],
                    acc_ps[0:mc5, lane5, 0:250],
                    1.0 / TSC, spn_sb[0:mc5, 0, 1, c0:c0 + 250], MU, MU)
                vector.tensor_reduce(rr_sb[0:mc5, 5 + half, 1:2],
                                     tmpb_sb[0:mc5, 1, c0:c0 + 250],
                                     AXX, AD).then_inc(rsem, 1)

        @block.scalar
        def _(scalar):
            scalar.wait_ge(dsem, 16)
            scalar.wait_ge(hsem, 16)
            d0, p0, a0 = DG_SPLIT[0]
            if a0:
                dg_build(0, d0 + p0, NPAIR,
                         lambda o, i, s: scalar.mul(o, i, s), dgsem)

            def spat_copy(ct):
                scalar.wait_ge(psem, ct + 1)
                if ct >= 2:
                    scalar.wait_ge(upsem, 2 * (ct - 1))  # spatj slot free
                scalar.mul(spatj_sb[0:128, ct % 2, 0:CT_SZ[ct]],
                           spat_ps[0:128, 0:CT_SZ[ct]],
                           1.0 / FSC).then_inc(scsem, 1)

            def up_copy(ct, b):
                mc = CT_SZ[ct]
                us = ct % 2
                scalar.wait_ge(upsem, 4 * ct + 2 * b + 1)
                if ct >= 2 and b == 0:
                    scalar.wait_ge(chsem, 2)  # up_sb slot0 free after units 0,1
                scalar.activation(up_sb[0:mc, us, b, 0, 0:375],
                                  up_ps[0:mc, b, 0, 0:375], CPY)
                scalar.activation(up_sb[0:mc, us, b, 1, 0:374],
                                  up_ps[0:mc, b, 0, 1:375], CPY)
                scalar.wait_ge(upsem, 4 * ct + 2 * b + 2)
                scalar.activation(up_sb[0:mc, us, b, 0, 375:750],
                                  up_ps[0:mc, b, 1, 0:375], CPY)
                scalar.activation(up_sb[0:mc, us, b, 1, 374:749],
                                  up_ps[0:mc, b, 1, 0:375],
                                  CPY).then_inc(ucsem, 1)

            def accums(u):
                ct, b = u // 2, u % 2
                mc = CT_SZ[ct]
                us = u % 2
                scalar.wait_ge(tsem, u + 1)
                scalar.activation(junk_sb[0:mc, us, :], tmpa_sb[0:mc, us, :],
                                  EXP,
                                  accum_out=rr_sb[0:mc, u, 0:1]).then_inc(asem, 1)

            spat_copy(0)
            up_copy(0, 0)
            up_copy(0, 1)
            accums(0)
            spat_copy(1)
            up_copy(1, 0)
            up_copy(1, 1)
            spat_copy(2)
            up_copy(2, 0)
            up_copy(2, 1)
            accums(1)
            accums(2)
            accums(3)
            accums(4)
            for half in range(2):
                mc5 = CT_SZ[2]
                c0 = 250 * half
                scalar.wait_ge(tsem, 6 + half)
                scalar.activation(junk_sb[0:mc5, 1, c0:c0 + 250],
                                  tmpa_sb[0:mc5, 1, c0:c0 + 250], EXP,
                                  accum_out=rr_sb[0:mc5, 5 + half,
                                                  0:1]).then_inc(asem, 1)
    return nc


_NC_CACHE = {}


def _host_prep(inputs):
    img = np.asarray(inputs["batched_image"], dtype=np.float32)
    spikes = np.asarray(inputs["batched_spikes"], dtype=np.float32)
    em = np.asarray(inputs["eye_movements"]).astype(np.int64)
    tmask = np.asarray(inputs["time_mask"], dtype=np.float32)
    sel = np.asarray(inputs["forward_sel"]).astype(np.int64)
    fw = np.asarray(inputs["forward_weights"], dtype=np.float32)
    F = np.asarray(inputs["stacked_flat_spat_filters"], dtype=np.float32)
    tcf = np.asarray(inputs["stacked_timecourse_filters"], dtype=np.float32)
    fbg = np.asarray(inputs["precomputed_feedback_gensig"], dtype=np.float32)
    histf = np.asarray(inputs["precomputed_history_frames"], dtype=np.float32)

    # jitter on host (pure gather, exact)
    jit = np.zeros((B, NJ, H, W), dtype=np.float32)
    for b in range(B):
        for f in range(NJ):
            dy, dx = int(em[b, f, 0]), int(em[b, f, 1])
            ys, xs = max(0, -dy), max(0, -dx)
            ye, xe = min(H, H - dy), min(W, W - dx)
            if ye > ys and xe > xs:
                jit[b, f, ys:ye, xs:xe] = img[b, ys + dy:ye + dy, xs + dx:xe + dx]
    jitT = jit.reshape(B, NJ, P).transpose(0, 2, 1)                 # (B,P,NJ)
    # pixel index = ch*2048 + q*128 + p
    jitp = np.zeros((B, NCH, PT, QPC, NJP), dtype=np.float32)
    jitp[..., 0:NJ] = jitT.reshape(B, NCH, QPC, PT, NJ).transpose(0, 1, 3, 2, 4)
    jitc = jitp.astype(F8)
    # jtd per batch group: (PT, NCH, QPC, BL, NJP)
    jt_h = [np.ascontiguousarray(
        np.stack([jitc[BL * bg + b] for b in range(BL)], axis=3)
        .transpose(1, 0, 2, 3, 4))
        for bg in range(GB)]

    FTf = F.T * np.float32(FSC)                                     # (P,C)
    ftc_h = {}
    for cg in range(GC):
        for ct in range(3):
            mc, off = CT_SZ[ct], CT_OFF[ct]
            X = FTf[:, cg * CL + off:cg * CL + off + mc]
            ftc_h[(cg, ct)] = np.ascontiguousarray(
                X.reshape(NCH, QPC, PT, mc).transpose(2, 0, 1, 3)
                .reshape(PT, NCH, QPC * mc)).astype(F8)

    # upsample mixing matrix M[f,t]
    Mm = np.zeros((B, NF, NB), dtype=np.float32)
    tix = np.arange(NB)
    for b in range(B):
        np.add.at(Mm[b], (sel[b, :, 0], tix), fw[b, :, 0])
        np.add.at(Mm[b], (sel[b, :, 1], tix), fw[b, :, 1])
    Mmb = Mm.astype(BF)

    mv = tmask * np.float32(MAGIC)                                  # (B,500)
    with np.errstate(divide="ignore"):
        lmv = np.log(mv).astype(np.float32)
    spn_all = -(spikes[:, :, K:] * mv[:, None, :])                  # (B,C,500)
    fb5 = fbg[:, :, :TO]
    fbl_all = fb5 + lmv[:, None, :]                                 # fb + log(mv)
    # host part of linear term: sum_{c,t} spn*fb per batch
    hconst = np.einsum('bct,bct->b', spn_all.astype(np.float64), fb5.astype(np.float64))
    histb = histf.astype(BF)                                        # (B,NH,C)
    identity = np.eye(128, dtype=np.float32).astype(BF)
    tcf_s = tcf * np.float32(TSC)

    in_maps = []
    for i in range(8):
        bg, cg = i // GC, i % GC
        bs = slice(BL * bg, BL * (bg + 1))
        cs = slice(CL * cg, CL * (cg + 1))
        fbp = np.zeros((3, 128, BL, TO), dtype=BF)
        spnp = np.zeros((3, 128, BL, TO), dtype=BF)
        for ct in range(3):
            mc, off = CT_SZ[ct], CT_OFF[ct]
            for b in range(BL):
                fbp[ct, 0:mc, b, :] = fbl_all[BL * bg + b,
                                              cg * CL + off:cg * CL + off + mc, :].astype(BF)
                spnp[ct, 0:mc, b, :] = spn_all[BL * bg + b,
                                               cg * CL + off:cg * CL + off + mc, :].astype(BF)
        auxhp = np.zeros((NH, 2200), dtype=BF)
        auxhp[:, 0:700] = histb[bs][:, :, cs].transpose(1, 0, 2).reshape(NH, 700)
        auxhp[:, 700:2200] = Mmb[bs][:, 0:NH].transpose(1, 0, 2).reshape(NH, 1500)
        auxjp = np.zeros((128, 2, NB), dtype=BF)
        auxjp[0:NJ] = Mmb[bs][:, NH:NF].transpose(1, 0, 2)
        auxjp[64:64 + NJ] = auxjp[0:NJ]
        tfp = np.zeros((128, 3, K), dtype=np.float32)
        for ct in range(3):
            mc, off = CT_SZ[ct], CT_OFF[ct]
            tfp[0:mc, ct] = tcf_s[cg * CL + off:cg * CL + off + mc]
        in_maps.append({
            "jtd": jt_h[bg],
            "ft0": ftc_h[(cg, 0)],
            "ft1": ftc_h[(cg, 1)],
            "ft2": ftc_h[(cg, 2)],
            "auxh": auxhp,
            "auxj": auxjp,
            "fbd": fbp,
            "spnd": spnp,
            "tfd": tfp,
            "ident": identity,
        })
    return in_maps, hconst


def kernel(**inputs) -> np.ndarray:
    in_maps, hconst = _host_prep(inputs)
    if "nc" not in _NC_CACHE:
        _NC_CACHE["nc"] = _build_nc()
    nc = _NC_CACHE["nc"]

    if os.environ.get("KTRACE"):
        res = run_bass_kernel_spmd(
            nc, in_maps, core_ids=list(range(8)), trace=True,
            trace_cores=[0], tmpdir=os.environ.get("KTRACE_DIR") or None)
        kernel.last_results = res
    else:
        res = run_bass_kernel_spmd(nc, in_maps, core_ids=list(range(8)))
    out = np.array(hconst, dtype=np.float64)
    for i in range(8):
        bg = i // GC
        pr = res.results[i]["part"]
        for u in range(NU):
            ct, b = u // 2, u % 2
            mc = CT_SZ[ct]
            u1 = u + 2 if u == NU - 1 else u + 1
            out[BL * bg + b] += pr[0:mc, u:u1, :].sum(dtype=np.float64)
    return out.astype(np.float32)
